# revision 8
# baseline (speedup 1.0000x reference)
"""AtomicConv GNN message passing kernel for 8 TRN2 NeuronCores.

out[n, t*K+k] = sum_{e: dst[e]=n} (feat[src[e]] == ftu[t]) * he[e, k]
with the reference's raw (K,E,1)->(E,K) reshape quirk:
  he[e, k] = f_{e//ME}( d[K*(e%ME)+k] ),  ME = E//K
  f_j(x) = exp(-s_j (x-mu_j)^2) * 0.5*(cos(pi*x/c_j)+1) * (x <= c_j)

Sharding strategy: edge-parallel, partitioned by destination-node range
(2500 nodes per core, no collective needed), with edges grouped into
(node-block, type-pair) chunks of 128-edge tiles.  Each core computes the
radial basis (ACT: sin/square/exp; DVE: products & masks), builds
destination one-hots (bulk DVE compares), and performs the segment-sum as
PSUM-accumulated TensorE matmuls over a 10-stage software pipeline,
writing its node range of the output.
"""
import sys
for p in ('/opt/trn_rl_repo', '/root/.axon_site/_ro/trn_rl_repo'):
    if p not in sys.path:
        sys.path.append(p)
from contextlib import ExitStack

import numpy as np
import ml_dtypes

import concourse.bass as bass
import concourse.bacc as bacc
import concourse.mybir as mybir
from concourse.bass_utils import run_bass_kernel_spmd

F32 = mybir.dt.float32
BF16 = mybir.dt.bfloat16
BF16_NP = ml_dtypes.bfloat16

NCORE = 8
NPC = 2500          # real nodes per core
NPAD = 2560         # padded node range: 20 blocks x 128
NBLK = 20
PI = float(np.pi)
PAD_DIST = 50.0
NRANGE = 10


def shard_inputs(feat, distances, radial_params, features_to_use, src, dst):
    feat = np.asarray(feat, np.float32).reshape(-1)
    d = np.asarray(distances, np.float32).reshape(-1)
    rp = np.asarray(radial_params, np.float32)
    ftu = np.asarray(features_to_use, np.float32).reshape(-1)
    src = np.asarray(src).reshape(-1)
    dst = np.asarray(dst).reshape(-1)
    T = ftu.shape[0]
    K = rp.shape[0]
    E = d.shape[0]
    ME = E // K
    assert T % 2 == 0 and E % K == 0
    NTP = T // 2

    fsrc = feat[src]
    eis, ets = [], []
    for t in range(T):
        sel = np.nonzero(fsrc == ftu[t])[0]
        eis.append(sel)
        ets.append(np.full(sel.shape, t, np.int64))
    ei = np.concatenate(eis)            # original edge ids, typed only
    et = np.concatenate(ets)
    edst = dst[ei].astype(np.int64)

    core = edst // NPC
    dstl = edst - core * NPC
    blk = dstl >> 7
    nl = dstl & 127
    tp = et >> 1
    pb = (et & 1).astype(np.float32)
    NCHUNK = NBLK * NTP
    chunk = blk * NTP + tp

    counts = np.zeros((NCORE, NCHUNK), np.int64)
    np.add.at(counts, (core, chunk), 1)
    ntiles = np.maximum(1, -(-counts.max(axis=0) // 128))     # per chunk
    tile_base = np.zeros(NCHUNK, np.int64)
    tile_base[1:] = np.cumsum(ntiles)[:-1]
    S = int(ntiles.sum())

    # rank of each edge within its (core, chunk) group
    key = core * NCHUNK + chunk
    order = np.argsort(key, kind='stable')
    sorted_key = key[order]
    starts = np.searchsorted(sorted_key, np.arange(NCORE * NCHUNK))
    rank_sorted = np.arange(len(order)) - starts[sorted_key]
    rank = np.empty(len(order), np.int64)
    rank[order] = rank_sorted
    slot_tile = tile_base[chunk] + (rank >> 7)
    slot_p = rank & 127

    # per-edge payload under the reshape quirk
    kk = ei // ME                       # radial kernel index per edge
    er = ei % ME
    dvec = d[(K * er)[:, None] + np.arange(K)[None, :]]     # (ne, K)
    mu_e = rp[kk, 1]
    negs_e = -rp[kk, 2]
    c_e = rp[kk, 0]

    d3_a = np.full((NCORE, 128, S, K), PAD_DIST, np.float32)
    mu_a = np.zeros((NCORE, 128, S), np.float32)
    negs_a = np.zeros((NCORE, 128, S), np.float32)
    invc_a = np.full((NCORE, 128, S), 1.0, np.float32)
    nl_a = np.zeros((NCORE, 128, S), np.float32)
    pb0_a = np.zeros((NCORE, 128, S), np.float32)
    pb1_a = np.zeros((NCORE, 128, S), np.float32)
    d3_a[core, slot_p, slot_tile] = dvec
    mu_a[core, slot_p, slot_tile] = mu_e
    negs_a[core, slot_p, slot_tile] = negs_e
    invc_a[core, slot_p, slot_tile] = PI / c_e
    nl_a[core, slot_p, slot_tile] = nl
    pb0_a[core, slot_p, slot_tile] = 0.5 * (1.0 - pb)   # 0.5 factor folded in
    pb1_a[core, slot_p, slot_tile] = 0.5 * pb

    iota_a = np.broadcast_to(np.arange(128, dtype=np.float32), (128, 128)).copy()
    uniform_c = float(rp[0, 0]) if np.all(rp[:, 0] == rp[0, 0]) else None

    in_maps = []
    for c in range(NCORE):
        m = {
            "d3": d3_a[c].reshape(128, S * K),
            "mu": mu_a[c], "negs": negs_a[c],
            "nl": nl_a[c].astype(BF16_NP),
            "pb0": pb0_a[c].astype(BF16_NP),
            "pb1": pb1_a[c].astype(BF16_NP),
            "iota": iota_a.astype(BF16_NP),
        }
        if uniform_c is None:
            m["invc"] = invc_a[c]
        in_maps.append(m)
    sched = [(int(c // NTP), int(c % NTP), int(tile_base[c]), int(ntiles[c]))
             for c in range(NCHUNK)]
    return in_maps, sched, S, K, NTP, uniform_c


def build_nc(S, K, NTP, sched, uniform_c):
    """One program for all 8 cores (SPMD, no collectives)."""
    SK = S * K
    uni = uniform_c is not None
    nc = bacc.Bacc(None, target_bir_lowering=False, debug=False)
    p_d3 = nc.declare_dram_parameter("d3", [128, SK], F32, isOutput=False)
    p_mu = nc.declare_dram_parameter("mu", [128, S], F32, isOutput=False)
    p_negs = nc.declare_dram_parameter("negs", [128, S], F32, isOutput=False)
    p_nl = nc.declare_dram_parameter("nl", [128, S], BF16, isOutput=False)
    p_pb0 = nc.declare_dram_parameter("pb0", [128, S], BF16, isOutput=False)
    p_pb1 = nc.declare_dram_parameter("pb1", [128, S], BF16, isOutput=False)
    p_iota = nc.declare_dram_parameter("iota", [128, 128], BF16, isOutput=False)
    if not uni:
        p_invc = nc.declare_dram_parameter("invc", [128, S], F32, isOutput=False)
    p_out = nc.declare_dram_parameter("outT", [128, NPAD], F32, isOutput=True)

    NBANK = (NBLK + 3) // 4   # 4 blocks of 128 nodes per psum bank

    with ExitStack() as es:
        block = es.enter_context(nc.Block())

        def sem(name):
            return es.enter_context(nc.semaphore(name))

        def sbuf(name, shape, dt):
            return es.enter_context(nc.sbuf_tensor(name, shape, dt))

        dma_in = sem("dma_in"); ve = sem("ve"); ac = sem("ac")
        pe = sem("pe"); outc = sem("outc"); dma_out = sem("dma_out")

        sb_d3 = sbuf("sb_d3", [128, SK], F32)     # also reused for the output
        sb_mu = sbuf("sb_mu", [128, S], F32)
        sb_negs = sbuf("sb_negs", [128, S], F32)
        sb_nl = sbuf("sb_nl", [128, S], BF16)
        sb_pb0 = sbuf("sb_pb0", [128, S], BF16)
        sb_pb1 = sbuf("sb_pb1", [128, S], BF16)
        sb_iota = sbuf("sb_iota", [128, 128], BF16)
        sb_invc = None if uni else sbuf("sb_invc", [128, S], F32)
        sb_uw = sbuf("sb_uw", [128, SK], F32)     # u, then w
        sb_vx = sbuf("sb_vx", [128, SK], F32)     # v, then x
        sb_e1 = sbuf("sb_e1", [128, SK], BF16)
        sb_c1 = sbuf("sb_c1", [128, SK], BF16)
        sb_he = sbuf("sb_he", [128, SK], BF16)
        sb_he2 = sbuf("sb_he2", [128, S, 2 * K], BF16)
        sb_oh = sbuf("sb_oh", [128, S, 128], BF16)
        psums = [es.enter_context(nc.psum_tensor(f"psum{q}", [128, 512], F32))
                 for q in range(NBANK)]

        d3v = sb_d3[:, :].rearrange("p (s k) -> p s k", k=K)
        mu_rep = sb_mu[:, :].unsqueeze(-1).to_broadcast([128, S, K])
        negs_rep = sb_negs[:, :].unsqueeze(-1).to_broadcast([128, S, K])
        pb0_rep = sb_pb0[:, :].unsqueeze(-1).to_broadcast([128, S, K])
        pb1_rep = sb_pb1[:, :].unsqueeze(-1).to_broadcast([128, S, K])
        iota_rep = sb_iota[:, :].unsqueeze(1).to_broadcast([128, S, 128])
        nl_rep = sb_nl[:, :].unsqueeze(-1).to_broadcast([128, S, 128])
        invc_rep = (None if uni else
                    sb_invc[:, :].unsqueeze(-1).to_broadcast([128, S, K]))

        uwv = sb_uw[:, :].rearrange("p (s k) -> p s k", k=K)
        vxv = sb_vx[:, :].rearrange("p (s k) -> p s k", k=K)
        e1v = sb_e1[:, :].rearrange("p (s k) -> p s k", k=K)
        c1v = sb_c1[:, :].rearrange("p (s k) -> p s k", k=K)
        hev = sb_he[:, :].rearrange("p (s k) -> p s k", k=K)
        out_view = sb_d3[:, 0:NPAD]               # reuse d3 buffer for output

        dmas = [(sb_d3, p_d3), (sb_mu, p_mu), (sb_negs, p_negs), (sb_nl, p_nl),
                (sb_pb0, p_pb0), (sb_pb1, p_pb1), (sb_iota, p_iota)]
        if not uni:
            dmas.append((sb_invc, p_invc))
        n_dma_in = len(dmas) * 16

        vemark = {}

        @block.sync
        def _(sync):
            for sb, pr in dmas:
                sync.dma_start(out=sb[:, :], in_=pr[:, :]).then_inc(dma_in, 16)
            sync.wait_ge(outc, NBANK)
            sync.dma_start(out=p_out[:, :], in_=out_view).then_inc(dma_out, 16)
            sync.wait_ge(dma_out, 16)

        @block.vector
        def _(vector):
            cnt = [0]

            def fin(inst, mark=None):
                cnt[0] += 1
                inst.then_inc(ve, 1)
                if mark:
                    vemark[mark] = cnt[0]

            vector.wait_ge(dma_in, n_dma_in)
            # u = d - mu
            fin(vector.tensor_tensor(out=uwv, in0=d3v, in1=mu_rep,
                                     op=mybir.AluOpType.subtract), "u")
            # w = v * (-s)   (v = u^2 from ACT; overwrites u)
            vector.wait_ge(ac, 1)
            fin(vector.tensor_tensor(out=uwv, in0=vxv, in1=negs_rep,
                                     op=mybir.AluOpType.mult), "w")
            # x = clamp(pi/2 - d*pi/c, >= -pi/2)  (overwrites v)
            vector.wait_ge(ve, vemark["w"])
            if uni:
                fin(vector.tensor_scalar(out=sb_vx[:, :], in0=sb_d3[:, :],
                                         scalar1=-PI / float(uniform_c),
                                         scalar2=PI / 2.0,
                                         op0=mybir.AluOpType.mult,
                                         op1=mybir.AluOpType.add), "x1")
            else:
                fin(vector.scalar_tensor_tensor(out=vxv, in0=d3v, scalar=-1.0,
                                                in1=invc_rep,
                                                op0=mybir.AluOpType.mult,
                                                op1=mybir.AluOpType.mult), "x0")
                vector.wait_ge(ve, vemark["x0"])
                fin(vector.tensor_scalar(out=sb_vx[:, :], in0=sb_vx[:, :],
                                         scalar1=PI / 2.0, scalar2=None,
                                         op0=mybir.AluOpType.add), "x1")
            vector.wait_ge(ve, vemark["x1"])
            fin(vector.tensor_scalar(out=sb_vx[:, :], in0=sb_vx[:, :],
                                     scalar1=-PI / 2.0, scalar2=None,
                                     op0=mybir.AluOpType.max), "x")
            # he = (c1 + 1) * e1      (c1: ac>=2, e1: ac>=3)
            vector.wait_ge(ac, 3)
            fin(vector.scalar_tensor_tensor(out=hev, in0=c1v, scalar=1.0,
                                            in1=e1v,
                                            op0=mybir.AluOpType.add,
                                            op1=mybir.AluOpType.mult), "he")
            vector.wait_ge(ve, vemark["he"])
            fin(vector.tensor_tensor(out=sb_he2[:, :, 0:K], in0=hev, in1=pb0_rep,
                                     op=mybir.AluOpType.mult), "he2a")
            fin(vector.tensor_tensor(out=sb_he2[:, :, K:2 * K], in0=hev,
                                     in1=pb1_rep,
                                     op=mybir.AluOpType.mult), "he2b")
            fin(vector.tensor_tensor(out=sb_oh[:, :, :], in0=iota_rep, in1=nl_rep,
                                     op=mybir.AluOpType.is_equal), "oh")

        @block.scalar
        def _(scalar):
            scalar.wait_ge(dma_in, n_dma_in)
            scalar.wait_ge(ve, vemark["u"])
            scalar.activation(out=vxv, in_=uwv,
                              func=mybir.ActivationFunctionType.Square
                              ).then_inc(ac, 1)                       # ac=1 (v)
            scalar.wait_ge(ve, vemark["x"])
            scalar.activation(out=sb_c1[:, :], in_=sb_vx[:, :],
                              func=mybir.ActivationFunctionType.Sin
                              ).then_inc(ac, 1)                       # ac=2 (c1)
            scalar.wait_ge(ve, vemark["w"])
            scalar.activation(out=e1v, in_=uwv,
                              func=mybir.ActivationFunctionType.Exp
                              ).then_inc(ac, 1)                       # ac=3 (e1)
            # after matmuls: copy psum -> sbuf (into the retired d3 buffer)
            scalar.wait_ge(pe, 1)
            for q in range(NBANK):
                scalar.activation(out=out_view[:, q * 512:(q + 1) * 512],
                                  in_=psums[q][:, :],
                                  func=mybir.ActivationFunctionType.Copy
                                  ).then_inc(outc, 1)

        @block.tensor
        def _(tensor):
            tensor.wait_ge(ve, vemark["oh"])
            last = None
            for q in range(NBANK):
                for tp in range(4):
                    grp = [c for c in sched if c[0] // 4 == q and c[1] == tp]
                    n_mm = sum(c[3] for c in grp)
                    i_mm = 0
                    for (b, _tp, tbase, nt) in grp:
                        for i in range(nt):
                            tile = tbase + i
                            last = tensor.matmul(
                                psums[q][32 * tp:32 * tp + 32,
                                         (b % 4) * 128:(b % 4) * 128 + 128],
                                sb_he2[:, tile, :],
                                sb_oh[:, tile, :],
                                start=(i_mm == 0),
                                stop=(i_mm == n_mm - 1),
                                tile_position=(0, 32 * tp),
                            )
                            i_mm += 1
            last.then_inc(pe, 1)

    return nc


def run_gnn(feat, distances, radial_params, features_to_use, src, dst,
            trace=False, tmpdir=None):
    in_maps, sched, S, K, NTP, uniform_c = shard_inputs(
        feat, distances, radial_params, features_to_use, src, dst)
    nc = build_nc(S, K, NTP, sched, uniform_c)
    nc.compile()
    res = run_bass_kernel_spmd(nc, in_maps, core_ids=list(range(NCORE)),
                               trace=trace, tmpdir=tmpdir)
    n_nodes = np.asarray(feat).shape[0]
    TK = 2 * K * NTP
    out = np.zeros((n_nodes, TK), np.float32)
    for c in range(NCORE):
        outT = res.results[c]["outT"]          # [128, NPAD]
        out[c * NPC:(c + 1) * NPC, :] = outT.T[:NPC, :].astype(np.float32)
    return out, res


def shard_inputs_v4(feat, distances, radial_params, features_to_use, src, dst):
    feat = np.asarray(feat, np.float32).reshape(-1)
    d = np.asarray(distances, np.float32).reshape(-1)
    rp = np.asarray(radial_params, np.float32)
    ftu = np.asarray(features_to_use, np.float32).reshape(-1)
    src = np.asarray(src).reshape(-1)
    dst = np.asarray(dst).reshape(-1)
    T = ftu.shape[0]
    K = rp.shape[0]
    E = d.shape[0]
    ME = E // K
    NTP = T // 2

    uniform_c = float(rp[0, 0]) if np.all(rp[:, 0] == rp[0, 0]) else None
    fast = (uniform_c is not None and T % 2 == 0 and E % K == 0
            and float(d.max()) <= uniform_c and float(d.min()) >= 0.0
            and NBLK * NTP % 4 == 0)
    if not fast:
        return None

    fsrc = feat[src]
    eis, ets = [], []
    for t in range(T):
        sel = np.nonzero(fsrc == ftu[t])[0]
        eis.append(sel)
        ets.append(np.full(sel.shape, t, np.int64))
    ei = np.concatenate(eis)
    et = np.concatenate(ets)
    edst = dst[ei].astype(np.int64)

    core = edst // NPC
    dstl = edst - core * NPC
    blk = dstl >> 7
    nl = dstl & 127
    tp = et >> 1
    pb = (et & 1).astype(np.float32)
    NCHUNK = NBLK * NTP
    chunk = blk * NTP + tp

    counts = np.zeros((NCORE, NCHUNK), np.int64)
    np.add.at(counts, (core, chunk), 1)
    ntiles = np.maximum(1, -(-counts.max(axis=0) // 128))
    tile_base = np.zeros(NCHUNK, np.int64)
    tile_base[1:] = np.cumsum(ntiles)[:-1]
    S = int(ntiles.sum())

    key = core * NCHUNK + chunk
    order = np.argsort(key, kind='stable')
    sorted_key = key[order]
    starts = np.searchsorted(sorted_key, np.arange(NCORE * NCHUNK))
    rank_sorted = np.arange(len(order)) - starts[sorted_key]
    rank = np.empty(len(order), np.int64)
    rank[order] = rank_sorted
    slot_tile = tile_base[chunk] + (rank >> 7)
    slot_p = rank & 127

    kk = ei // ME
    er = ei % ME
    dvec = d[(K * er)[:, None] + np.arange(K)[None, :]]
    sq_e = np.sqrt(rp[kk, 2])
    dvs = (dvec - rp[kk, 1][:, None]) * sq_e[:, None]   # sqrt(s)*(d-mu)

    pad_d = 1.4925 * uniform_c
    d3_a = np.full((NCORE, 128, S, K), pad_d, np.float32)
    d3s_a = np.full((NCORE, 128, S, K), 50.0, np.float32)
    nl_a = np.zeros((NCORE, 128, S), np.float32)
    pb0_a = np.zeros((NCORE, 128, S), np.float32)
    pb1_a = np.zeros((NCORE, 128, S), np.float32)
    d3_a[core, slot_p, slot_tile] = dvec
    d3s_a[core, slot_p, slot_tile] = dvs
    nl_a[core, slot_p, slot_tile] = nl
    pb0_a[core, slot_p, slot_tile] = 0.5 * (1.0 - pb)
    pb1_a[core, slot_p, slot_tile] = 0.5 * pb

    iota_a = np.broadcast_to(np.arange(128, dtype=np.float32), (128, 128)).copy()
    pb0e = np.repeat(pb0_a, K, axis=2).astype(BF16_NP)   # [NCORE,128,S*K]
    pb1e = np.repeat(pb1_a, K, axis=2).astype(BF16_NP)

    in_maps = []
    for c in range(NCORE):
        in_maps.append({
            "d3": d3_a[c].reshape(128, S * K),
            "d3s": d3s_a[c].reshape(128, S * K),
            "nl": nl_a[c],
            "pb0e": pb0e[c], "pb1e": pb1e[c],
            "iota": iota_a.astype(BF16_NP),
            "hpi": np.full((128, 1), PI / 2.0, np.float32),
        })
    sched = [(int(c // NTP), int(c % NTP), int(tile_base[c]), int(ntiles[c]))
             for c in range(NCHUNK)]
    return in_maps, sched, S, K, NTP, uniform_c


def build_nc_v4(S, K, NTP, sched, uniform_c):
    SK = S * K
    nc = bacc.Bacc(None, target_bir_lowering=False, debug=False)
    p_d3 = nc.declare_dram_parameter("d3", [128, SK], F32, isOutput=False)
    p_d3s = nc.declare_dram_parameter("d3s", [128, SK], F32, isOutput=False)
    p_nl = nc.declare_dram_parameter("nl", [128, S], F32, isOutput=False)
    p_pb0e = nc.declare_dram_parameter("pb0e", [128, SK], BF16, isOutput=False)
    p_pb1e = nc.declare_dram_parameter("pb1e", [128, SK], BF16, isOutput=False)
    p_iota = nc.declare_dram_parameter("iota", [128, 128], BF16, isOutput=False)
    p_hpi = nc.declare_dram_parameter("hpi", [128, 1], F32, isOutput=False)
    p_out = nc.declare_dram_parameter("outT", [128, NPAD], F32, isOutput=True)

    NBANK = NBLK // 4
    # tile ranges for the DVE/ACT pipeline
    rb = [round(S * r / NRANGE) for r in range(NRANGE + 1)]
    ranges = [(rb[r], rb[r + 1]) for r in range(NRANGE)]
    # bank tile spans (chunk ids are contiguous per bank)
    bank_span = []
    for q in range(NBANK):
        lo = sched[16 * q][2]
        hi_c = sched[16 * q + 15]
        bank_span.append((lo, hi_c[2] + hi_c[3]))
    # last pipeline range needed per bank
    bank_need_range = [max(r for r in range(NRANGE) if ranges[r][0] < hi)
                       for (lo, hi) in bank_span]

    OHB = 6                     # one-hot rotating buffer (in units of ranges)
    # last bank whose tile span covers range j (for oh slot-reuse gating)
    range_last_bank = [max(q for q in range(NBANK)
                           if bank_span[q][0] < ranges[j][1])
                       for j in range(NRANGE)]
    max_rt = max(b - a for a, b in ranges)

    with ExitStack() as es:
        block = es.enter_context(nc.Block())

        def sem(name):
            return es.enter_context(nc.semaphore(name))

        def sbuf(name, shape, dt):
            return es.enter_context(nc.sbuf_tensor(name, shape, dt))

        dma_in = sem("dma_in"); ve = sem("ve"); ac = sem("ac")
        pe = sem("pe"); outc = sem("outc"); dma_out = sem("dma_out")
        dma_d3 = [sem(f"dma_d3_{r}") for r in range(NRANGE)]
        dma_ds = [sem(f"dma_ds_{r}") for r in range(NRANGE)]
        dma_pb = [sem(f"dma_pb_{r}") for r in range(NRANGE)]

        sb_d3 = sbuf("sb_d3", [128, SK], F32)
        sb_d3s = sbuf("sb_d3s", [128, SK], F32)
        sb_nl = sbuf("sb_nl", [128, S], F32)
        sb_pb0e = sbuf("sb_pb0e", [128, SK], BF16)
        sb_pb1e = sbuf("sb_pb1e", [128, SK], BF16)
        sb_iota = sbuf("sb_iota", [128, 128], BF16)
        sb_hpi = sbuf("sb_hpi", [128, 1], F32)
        sb_v = sbuf("sb_v", [128, SK], F32)
        sb_e1 = sbuf("sb_e1", [128, SK], BF16)
        sb_c1 = sbuf("sb_c1", [128, SK], BF16)
        sb_ht = sbuf("sb_ht", [128, SK], BF16)
        sb_he2 = sbuf("sb_he2", [128, S, 2 * K], BF16)
        sb_oh = sbuf("sb_oh", [128, OHB, max_rt * 128], BF16)
        sb_out = sbuf("sb_out", [128, NPAD], F32)
        psums = [es.enter_context(nc.psum_tensor(f"psum{q}", [128, 512], F32))
                 for q in range(NBANK)]

        # DMA plan: small tensors first, then per-range d3/pb0e/pb1e
        small_dmas = [(sb_nl, p_nl), (sb_iota, p_iota), (sb_hpi, p_hpi)]
        n_small = len(small_dmas)
        # dma_in counts: small: 16 each; then per range r: 3 DMAs
        def dma_mark_small():
            return 16 * n_small

        def dma_mark_range(r):
            return 16 * n_small + 48 * (r + 1)

        vemark = {}
        acmark = {}

        @block.sync
        def _(sync):
            for sb, pr in small_dmas:
                sync.dma_start(out=sb[:, :], in_=pr[:, :]).then_inc(dma_in, 16)
            for r, (a, b) in enumerate(ranges):
                ka, kb = a * K, b * K
                sync.dma_start(out=sb_d3[:, ka:kb],
                               in_=p_d3[:, ka:kb]).then_inc(dma_d3[r], 16)
            for r, (a, b) in enumerate(ranges):
                ka, kb = a * K, b * K
                sync.dma_start(out=sb_d3s[:, ka:kb],
                               in_=p_d3s[:, ka:kb]).then_inc(dma_ds[r], 16)
            for r, (a, b) in enumerate(ranges):
                ka, kb = a * K, b * K
                sync.dma_start(out=sb_pb0e[:, ka:kb],
                               in_=p_pb0e[:, ka:kb]).then_inc(dma_pb[r], 16)
                sync.dma_start(out=sb_pb1e[:, ka:kb],
                               in_=p_pb1e[:, ka:kb]).then_inc(dma_pb[r], 16)
            # output: per bank as soon as copied
            for q in range(NBANK):
                sync.wait_ge(outc, q + 1)
                sync.dma_start(out=p_out[:, q * 512:(q + 1) * 512],
                               in_=sb_out[:, q * 512:(q + 1) * 512]
                               ).then_inc(dma_out, 16)
            sync.wait_ge(dma_out, 16 * NBANK)

        @block.vector
        def _(vector):
            cnt = [0]

            def fin(inst, mark=None):
                cnt[0] += 1
                inst.then_inc(ve, 1)
                if mark:
                    vemark[mark] = cnt[0]

            def emit_oh(r):
                a, b = ranges[r]
                nt = b - a
                par = r % OHB
                if r >= OHB:
                    vector.wait_ge(pe, range_last_bank[r - OHB] + 1)
                fin(vector.tensor_tensor(
                    out=sb_oh[:, par, 0:nt * 128].rearrange(
                        "p (t n) -> p t n", n=128),
                    in0=sb_iota[:, :].unsqueeze(1).to_broadcast([128, nt, 128]),
                    in1=sb_nl[:, a:b].unsqueeze(-1).to_broadcast([128, nt, 128]),
                    op=mybir.AluOpType.is_equal))
                vemark[f"oh{r}"] = cnt[0]

            def emit_chain(r):
                a, b = ranges[r]
                ka, kb = a * K, b * K
                # c1p1 = c1 + 1 (in place), needs Sin_r  (ac >= r+1)
                vector.wait_ge(ac, r + 1)
                fin(vector.tensor_scalar(out=sb_c1[:, ka:kb],
                                         in0=sb_c1[:, ka:kb],
                                         scalar1=1.0, scalar2=None,
                                         op0=mybir.AluOpType.add), f"c1p1{r}")
                # ht = c1p1 * e1, needs Exp_r  (ac >= NRANGE + 2r + 2)
                vector.wait_ge(ve, vemark[f"c1p1{r}"])
                vector.wait_ge(ac, NRANGE + 2 * r + 2)
                fin(vector.tensor_tensor(out=sb_ht[:, ka:kb],
                                         in0=sb_c1[:, ka:kb],
                                         in1=sb_e1[:, ka:kb],
                                         op=mybir.AluOpType.mult), f"ht{r}")
                vector.wait_ge(ve, vemark[f"ht{r}"])
                vector.wait_ge(dma_pb[r], 32)
                fin(vector.tensor_tensor(out=sb_he2[:, a:b, 0:K],
                                         in0=sb_ht[:, ka:kb].rearrange(
                                             "p (s k) -> p s k", k=K),
                                         in1=sb_pb0e[:, ka:kb].rearrange(
                                             "p (s k) -> p s k", k=K),
                                         op=mybir.AluOpType.mult), f"he2a{r}")
                fin(vector.tensor_tensor(out=sb_he2[:, a:b, K:2 * K],
                                         in0=sb_ht[:, ka:kb].rearrange(
                                             "p (s k) -> p s k", k=K),
                                         in1=sb_pb1e[:, ka:kb].rearrange(
                                             "p (s k) -> p s k", k=K),
                                         op=mybir.AluOpType.mult), f"he2b{r}")

            vector.wait_ge(dma_in, dma_mark_small())
            for r in range(NRANGE):
                emit_oh(r)
                if r >= 1:
                    emit_chain(r - 1)
            emit_chain(NRANGE - 1)

        @block.scalar
        def _(scalar):
            scalar.wait_ge(dma_in, dma_mark_small())
            # pass 1: all Sins (trig table loaded once)    ac = r+1
            for r, (a, b) in enumerate(ranges):
                ka, kb = a * K, b * K
                scalar.wait_ge(dma_d3[r], 16)
                scalar.activation(out=sb_c1[:, ka:kb], in_=sb_d3[:, ka:kb],
                                  func=mybir.ActivationFunctionType.Sin,
                                  bias=sb_hpi[:, 0:1],
                                  scale=-PI / float(uniform_c)).then_inc(ac, 1)
            # pass 2: Square + Exp per range (exp table)   ac = NRANGE+2r+1, +2
            for r, (a, b) in enumerate(ranges):
                ka, kb = a * K, b * K
                scalar.wait_ge(dma_ds[r], 16)
                scalar.activation(out=sb_v[:, ka:kb], in_=sb_d3s[:, ka:kb],
                                  func=mybir.ActivationFunctionType.Square
                                  ).then_inc(ac, 1)
                scalar.wait_ge(ac, NRANGE + 2 * r + 1)
                scalar.activation(out=sb_e1[:, ka:kb], in_=sb_v[:, ka:kb],
                                  func=mybir.ActivationFunctionType.Exp,
                                  scale=-1.0).then_inc(ac, 1)
            for q in range(NBANK):
                scalar.wait_ge(pe, q + 1)
                scalar.activation(out=sb_out[:, q * 512:(q + 1) * 512],
                                  in_=psums[q][:, :],
                                  func=mybir.ActivationFunctionType.Copy
                                  ).then_inc(outc, 1)

        @block.tensor
        def _(tensor):
            # range r tiles live in parity slot r%OHB of sb_oh
            def oh_ap(tile):
                r = next(i for i, (a, b) in enumerate(ranges)
                         if a <= tile < b)
                a, _ = ranges[r]
                i = tile - a
                return sb_oh[:, r % OHB, i * 128:(i + 1) * 128], r

            for q in range(NBANK):
                # wait for all DVE work covering this bank's tiles
                need_r = bank_need_range[q]
                tensor.wait_ge(ve, vemark[f"he2b{need_r}"])
                for tp in range(4):
                    grp = [c for c in sched if c[0] // 4 == q and c[1] == tp]
                    n_mm = sum(c[3] for c in grp)
                    i_mm = 0
                    for (bk, _tp, tbase, nt) in grp:
                        for i in range(nt):
                            tile = tbase + i
                            ohap, _r = oh_ap(tile)
                            last = tensor.matmul(
                                psums[q][32 * tp:32 * tp + 32,
                                         (bk % 4) * 128:(bk % 4) * 128 + 128],
                                sb_he2[:, tile, :],
                                ohap,
                                start=(i_mm == 0),
                                stop=(i_mm == n_mm - 1),
                                tile_position=(0, 32 * tp),
                            )
                            i_mm += 1
                last.then_inc(pe, 1)

    return nc


def run_gnn_v4(feat, distances, radial_params, features_to_use, src, dst,
               trace=False, tmpdir=None):
    r = shard_inputs_v4(feat, distances, radial_params, features_to_use,
                        src, dst)
    if r is None:
        return run_gnn(feat, distances, radial_params, features_to_use,
                       src, dst, trace=trace, tmpdir=tmpdir)
    in_maps, sched, S, K, NTP, uniform_c = r
    nc = build_nc_v4(S, K, NTP, sched, uniform_c)
    nc.compile()
    res = run_bass_kernel_spmd(nc, in_maps, core_ids=list(range(NCORE)),
                               trace=trace, tmpdir=tmpdir)
    n_nodes = np.asarray(feat).shape[0]
    TK = 2 * K * NTP
    out = np.zeros((n_nodes, TK), np.float32)
    for c in range(NCORE):
        outT = res.results[c]["outT"]
        out[c * NPC:(c + 1) * NPC, :] = outT.T[:NPC, :].astype(np.float32)
    return out, res


def shard_inputs_v5(feat, distances, radial_params, features_to_use, src, dst,
                    W=64, NG=10):
    """v5: W-wide node blocks, fp16 d3 + host-squared gaussian arg,
    one-hot strips split between GPSIMD local_scatter and DVE is_equal."""
    feat = np.asarray(feat, np.float32).reshape(-1)
    d = np.asarray(distances, np.float32).reshape(-1)
    rp = np.asarray(radial_params, np.float32)
    ftu = np.asarray(features_to_use, np.float32).reshape(-1)
    src = np.asarray(src).reshape(-1)
    dst = np.asarray(dst).reshape(-1)
    T = ftu.shape[0]
    K = rp.shape[0]
    E = d.shape[0]
    if K == 0 or E % K != 0 or T % 2 != 0:
        return None
    ME = E // K
    NTP = T // 2

    uniform_c = float(rp[0, 0]) if np.all(rp[:, 0] == rp[0, 0]) else None
    if (uniform_c is None or float(d.max()) > uniform_c
            or float(d.min()) < 0.0):
        return None
    NBLK = -(-NPC // W)
    if NBLK % 8 != 0:
        return None
    NBANK = NBLK // 8

    fsrc = feat[src]
    eis, ets = [], []
    for t in range(T):
        sel = np.nonzero(fsrc == ftu[t])[0]
        eis.append(sel)
        ets.append(np.full(sel.shape, t, np.int64))
    ei = np.concatenate(eis)
    et = np.concatenate(ets)
    edst = dst[ei].astype(np.int64)

    core = edst // NPC
    dstl = edst - core * NPC
    blk = dstl // W
    nl = dstl - blk * W
    tp = et >> 1
    pb = (et & 1).astype(np.float32)
    NCHUNK = NBLK * NTP
    chunk = blk * NTP + tp

    counts = np.zeros((NCORE, NCHUNK), np.int64)
    np.add.at(counts, (core, chunk), 1)
    ntiles = np.maximum(1, -(-counts.max(axis=0) // 128))
    S = int(ntiles.sum())
    if S % 2 == 1:              # even S so strip sizes stay even
        ntiles[-1] += 1
        S += 1

    tile_base = np.zeros(NCHUNK, np.int64)
    tile_base[1:] = np.cumsum(ntiles)[:-1]

    key = core * NCHUNK + chunk
    order = np.argsort(key, kind='stable')
    sorted_key = key[order]
    starts = np.searchsorted(sorted_key, np.arange(NCORE * NCHUNK))
    rank_sorted = np.arange(len(order)) - starts[sorted_key]
    rank = np.empty(len(order), np.int64)
    rank[order] = rank_sorted
    slot_tile = tile_base[chunk] + (rank >> 7)
    slot_p = rank & 127

    # one-hot strip grid (gpsimd local_scatter limit: nt*W*32 < 2^16)
    nt_cap = (2 ** 16 // 32 // W) & ~1
    NSTRIP = max(16, -(-S // nt_cap))
    nt_strip = -(-S // NSTRIP)
    nt_strip += nt_strip % 2
    NSTRIP = -(-S // nt_strip)
    rb = [min(S, i * nt_strip) for i in range(NSTRIP + 1)]
    dve_strips = [s_ for s_ in range(NSTRIP) if s_ % 5 == 4]
    gp_strips = [s_ for s_ in range(NSTRIP) if s_ % 5 != 4]
    NACT = 6
    fr = [0.0, 0.20, 0.40, 0.60, 0.78, 0.91, 1.0]
    cb = [int(round(S * f)) for f in fr]
    sin_groups = [[0], [1, 2], [3, 4, 5]]
    sin_of_chunk = [1, 2, 2, 3, 3, 3]

    # per-edge payload under the reshape quirk
    kk = ei // ME
    er = ei % ME
    dvec = d[(K * er)[:, None] + np.arange(K)[None, :]]     # (ne, K)
    dsq = (rp[kk, 2][:, None] * (dvec - rp[kk, 1][:, None]) ** 2)

    d3_a = np.full((NCORE, 128, S, K), 1.4925 * uniform_c, np.float32)
    dsq_a = np.full((NCORE, 128, S, K), 30000.0, np.float32)
    nl_a = np.zeros((NCORE, 128, S), np.float32)
    pb0_a = np.zeros((NCORE, 128, S), np.float32)
    pb1_a = np.zeros((NCORE, 128, S), np.float32)
    ohidx_a = np.full((NCORE, 128, S), -1, np.int16)
    d3_a[core, slot_p, slot_tile] = dvec
    dsq_a[core, slot_p, slot_tile] = dsq
    nl_a[core, slot_p, slot_tile] = nl
    pb0_a[core, slot_p, slot_tile] = 1.0 - pb
    pb1_a[core, slot_p, slot_tile] = pb
    strip_of_slot = np.minimum(slot_tile // nt_strip, NSTRIP - 1)
    ohidx_a[core, slot_p, slot_tile] = (
        (slot_tile - strip_of_slot * nt_strip) * W + nl).astype(np.int16)

    iota_a = np.broadcast_to(np.arange(W, dtype=np.float32), (128, W)).copy()
    pb0e = np.repeat(pb0_a, K, axis=2).astype(BF16_NP)

    in_maps = []
    for c in range(NCORE):
        in_maps.append({
            "d3": d3_a[c].reshape(128, S * K).astype(np.float16),
            "dsq": dsq_a[c].reshape(128, S * K).astype(np.float16),
            "nl": nl_a[c].astype(BF16_NP),
            "ohidx": ohidx_a[c],
            "pb0e": pb0e[c],
            "iota": iota_a.astype(BF16_NP),
            "hpi": np.full((128, 1), PI / 2.0, np.float32),
        })
    sched = [(int(c // NTP), int(c % NTP), int(tile_base[c]), int(ntiles[c]))
             for c in range(NCHUNK)]
    plan = dict(S=S, K=K, NTP=NTP, W=W, NBLK=NBLK, NBANK=NBANK,
                sched=sched, rb=rb, cb=cb, NSTRIP=NSTRIP, NACT=NACT,
                dve_strips=dve_strips, gp_strips=gp_strips,
                sin_groups=sin_groups, sin_of_chunk=sin_of_chunk,
                nt_strip=nt_strip, uniform_c=uniform_c)
    return in_maps, plan


def build_nc_v5(plan):
    S = plan["S"]; K = plan["K"]; NTP = plan["NTP"]; W = plan["W"]
    NBLK = plan["NBLK"]; NBANK = plan["NBANK"]; sched = plan["sched"]
    rb = plan["rb"]; cb = plan["cb"]; NSTRIP = plan["NSTRIP"]
    NACT = plan["NACT"]; nt_strip = plan["nt_strip"]
    dve_strips = plan["dve_strips"]; gp_strips = plan["gp_strips"]
    sin_groups = plan["sin_groups"]; sin_of_chunk = plan["sin_of_chunk"]
    uniform_c = plan["uniform_c"]
    SK = S * K
    NPADW = NBLK * W
    FP16 = mybir.dt.float16
    I16 = mybir.dt.int16

    nc = bacc.Bacc(None, target_bir_lowering=False, debug=False)
    p_d3 = nc.declare_dram_parameter("d3", [128, SK], FP16, isOutput=False)
    p_dsq = nc.declare_dram_parameter("dsq", [128, SK], FP16, isOutput=False)
    p_nl = nc.declare_dram_parameter("nl", [128, S], BF16, isOutput=False)
    p_ohidx = nc.declare_dram_parameter("ohidx", [128, S], I16, isOutput=False)
    p_pb0e = nc.declare_dram_parameter("pb0e", [128, SK], BF16, isOutput=False)
    p_iota = nc.declare_dram_parameter("iota", [128, W], BF16, isOutput=False)
    p_hpi = nc.declare_dram_parameter("hpi", [128, 1], F32, isOutput=False)
    p_out = nc.declare_dram_parameter("outT", [128, NPADW], BF16, isOutput=True)


    with ExitStack() as es:
        block = es.enter_context(nc.Block())

        def sem(name):
            return es.enter_context(nc.semaphore(name))

        def sbuf(name, shape, dt):
            return es.enter_context(nc.sbuf_tensor(name, shape, dt))

        dma_hd = sem("dma_hd")
        dma_d3 = [sem(f"dma_d3_{j}") for j in range(NACT)]
        dma_dq = [sem(f"dma_dq_{j}") for j in range(NACT)]
        dma_pb = [sem(f"dma_pb_{j}") for j in range(NACT)]
        sin_s = sem("sin_s"); exp_s = sem("exp_s")
        ve = sem("ve"); pool_s = sem("pool_s")
        pe = sem("pe"); outc = sem("outc"); dma_out = sem("dma_out")

        sb_d3 = sbuf("sb_d3", [128, SK], FP16)
        sb_dsq = sbuf("sb_dsq", [128, SK], FP16)
        sb_c1 = sbuf("sb_c1", [128, SK], BF16)      # sin result, then ht
        sb_e1 = sbuf("sb_e1", [128, SK], BF16)
        sb_pb0e = sbuf("sb_pb0e", [128, SK], BF16)
        sb_he2 = sbuf("sb_he2", [128, S, 2 * K], BF16)
        sb_oh = sbuf("sb_oh", [128, S * W], BF16)
        sb_nl = sbuf("sb_nl", [128, S], BF16)
        sb_ohidx = sbuf("sb_ohidx", [128, S], I16)
        sb_iota = sbuf("sb_iota", [128, W], BF16)
        sb_ones = sbuf("sb_ones", [128, nt_strip], BF16)
        sb_warm = sbuf("sb_warm", [128, 2], BF16)
        sb_warmi = sbuf("sb_warmi", [128, 2], I16)
        sb_hpi = sbuf("sb_hpi", [128, 1], F32)
        sb_out = sbuf("sb_out", [128, NPADW], BF16)
        psums = [es.enter_context(nc.psum_tensor(f"psum{q}", [128, 512], F32))
                 for q in range(NBANK)]

        vemark = {}

        # ---- DMA in: head, then per-act-chunk interleaved ----
        @block.sync
        def _(sync):
            sync.dma_start(out=sb_ohidx[:, :], in_=p_ohidx[:, :]).then_inc(dma_hd, 16)
            sync.dma_start(out=sb_nl[:, :], in_=p_nl[:, :]).then_inc(dma_hd, 16)
            sync.dma_start(out=sb_hpi[:, :], in_=p_hpi[:, :]).then_inc(dma_hd, 16)

            def dma_d3j(j):
                ka, kb = cb[j] * K, cb[j + 1] * K
                sync.dma_start(out=sb_d3[:, ka:kb],
                               in_=p_d3[:, ka:kb]).then_inc(dma_d3[j], 16)

            def dma_dqj(j):
                ka, kb = cb[j] * K, cb[j + 1] * K
                sync.dma_start(out=sb_dsq[:, ka:kb],
                               in_=p_dsq[:, ka:kb]).then_inc(dma_dq[j], 16)

            def dma_pbj(j):
                ka, kb = cb[j] * K, cb[j + 1] * K
                sync.dma_start(out=sb_pb0e[:, ka:kb],
                               in_=p_pb0e[:, ka:kb]).then_inc(dma_pb[j], 16)

            sync.dma_start(out=sb_iota[:, :], in_=p_iota[:, :]).then_inc(dma_hd, 16)
            dma_d3j(0)
            dma_dqj(0)
            dma_d3j(1)
            dma_pbj(0)
            dma_d3j(2)
            dma_dqj(1)
            dma_d3j(3)
            dma_pbj(1)
            dma_d3j(4)
            dma_dqj(2)
            dma_d3j(5)
            dma_pbj(2)
            for j in range(3, NACT):
                dma_dqj(j)
                dma_pbj(j)
            for q in range(NBANK):
                sync.wait_ge(outc, q + 1)
                sync.dma_start(out=p_out[:, q * 512:(q + 1) * 512],
                               in_=sb_out[:, q * 512:(q + 1) * 512]
                               ).then_inc(dma_out, 16)
            sync.wait_ge(dma_out, 16 * NBANK)

        # ---- DVE: memsets, late one-hot strips, he2 chains ----
        @block.vector
        def _(vector):
            cnt = [0]

            def fin(inst, mark=None):
                cnt[0] += 1
                inst.then_inc(ve, 1)
                if mark:
                    vemark[mark] = cnt[0]

            fin(vector.memset(sb_warmi[:, :], -1))
            fin(vector.memset(sb_ones[:, :], 1.0))          # ve=2: pool warmup
            waited_hd = [False]

            def emit_strip(s):
                if not waited_hd[0]:
                    vector.wait_ge(dma_hd, 64)
                    waited_hd[0] = True
                a, b = rb[s], rb[s + 1]
                nt = b - a
                fin(vector.tensor_tensor(
                    out=sb_oh[:, a * W:b * W].rearrange(
                        "p (t n) -> p t n", n=W),
                    in0=sb_iota[:, :].unsqueeze(1).to_broadcast([128, nt, W]),
                    in1=sb_nl[:, a:b].unsqueeze(-1).to_broadcast([128, nt, W]),
                    op=mybir.AluOpType.is_equal), f"oh{s}")

            def emit_chain(j):
                ka, kb = cb[j] * K, cb[j + 1] * K
                a, b = cb[j], cb[j + 1]
                vector.wait_ge(sin_s, sin_of_chunk[j])
                # c1p1h = 0.5*c1 + 0.5   (tensor_scalar, 4x mode)
                fin(vector.tensor_scalar(
                    out=sb_c1[:, ka:kb], in0=sb_c1[:, ka:kb],
                    scalar1=0.5, scalar2=0.5,
                    op0=mybir.AluOpType.mult, op1=mybir.AluOpType.add))
                vector.wait_ge(exp_s, j + 1)
                # ht = c1p1h * e1, in place over c1
                fin(vector.tensor_tensor(
                    out=sb_c1[:, ka:kb], in0=sb_c1[:, ka:kb],
                    in1=sb_e1[:, ka:kb], op=mybir.AluOpType.mult))
                vector.wait_ge(dma_pb[j], 16)
                fin(vector.tensor_tensor(
                    out=sb_he2[:, a:b, 0:K],
                    in0=sb_c1[:, ka:kb].rearrange("p (s k) -> p s k", k=K),
                    in1=sb_pb0e[:, ka:kb].rearrange("p (s k) -> p s k", k=K),
                    op=mybir.AluOpType.mult))
                fin(vector.tensor_tensor(
                    out=sb_he2[:, a:b, K:2 * K],
                    in0=sb_c1[:, ka:kb].rearrange("p (s k) -> p s k", k=K),
                    in1=sb_he2[:, a:b, 0:K],
                    op=mybir.AluOpType.subtract), f"he2{j}")

            for s in dve_strips:
                emit_strip(s)
            for j in range(NACT):
                emit_chain(j)

        # ---- GPSIMD: warmup (library load), early one-hot strips ----
        @block.gpsimd
        def _(gp):
            gp.wait_ge(ve, 2)
            gp.local_scatter(sb_warm[:, :], sb_ones[:, 0:2], sb_warmi[:, :],
                             channels=128, num_elems=2,
                             num_idxs=2).then_inc(pool_s, 1)
            gp.wait_ge(dma_hd, 16)
            for g in gp_strips:
                a, b = rb[g], rb[g + 1]
                nt = b - a
                gp.local_scatter(sb_oh[:, a * W:b * W], sb_ones[:, 0:nt],
                                 sb_ohidx[:, a:b], channels=128,
                                 num_elems=nt * W,
                                 num_idxs=nt).then_inc(pool_s, 1)

        # ---- ACT: split-phase sin/exp, then psum drains ----
        @block.scalar
        def _(scalar):
            scalar.wait_ge(ve, 2)
            scalar.activation(out=sb_warm[:, 0:1], in_=sb_ones[:, 0:1],
                              func=mybir.ActivationFunctionType.Sin)

            def emit_sin(grp):
                ka, kb = cb[grp[0]] * K, cb[grp[-1] + 1] * K
                scalar.wait_ge(dma_hd, 48)
                for j in grp:
                    scalar.wait_ge(dma_d3[j], 16)
                scalar.activation(out=sb_c1[:, ka:kb], in_=sb_d3[:, ka:kb],
                                  func=mybir.ActivationFunctionType.Sin,
                                  bias=sb_hpi[:, 0:1],
                                  scale=-PI / float(uniform_c)
                                  ).then_inc(sin_s, 1)

            def emit_exp(j):
                ka, kb = cb[j] * K, cb[j + 1] * K
                scalar.wait_ge(dma_dq[j], 16)
                scalar.activation(out=sb_e1[:, ka:kb], in_=sb_dsq[:, ka:kb],
                                  func=mybir.ActivationFunctionType.Exp,
                                  scale=-1.0).then_inc(exp_s, 1)

            emit_sin(sin_groups[0])
            emit_exp(0)
            for grp in sin_groups[1:]:
                emit_sin(grp)
            for j in range(1, NACT):
                emit_exp(j)
            for q in range(NBANK):
                scalar.wait_ge(pe, q + 1)
                scalar.activation(out=sb_out[:, q * 512:(q + 1) * 512],
                                  in_=psums[q][:, :],
                                  func=mybir.ActivationFunctionType.Copy
                                  ).then_inc(outc, 1)

        # ---- TensorE: psum-accumulated scatter matmuls ----
        gp_order = {g: i for i, g in enumerate(gp_strips)}

        @block.tensor
        def _(tensor):
            waited_strip = [-1]
            waited_chunk = [-1]

            def chunk_of(t):
                for j in range(NACT):
                    if t < cb[j + 1]:
                        return j
                return NACT - 1

            for q in range(NBANK):
                last = None
                for tp in range(NTP):
                    grp = [c for c in sched if c[0] // 8 == q and c[1] == tp]
                    n_mm = sum(c[3] for c in grp)
                    i_mm = 0
                    for (bk, _tp, tbase, nt) in grp:
                        for i in range(nt):
                            tile = tbase + i
                            s = min(tile // nt_strip, NSTRIP - 1)
                            if s > waited_strip[0]:
                                for s2 in range(waited_strip[0] + 1, s + 1):
                                    if s2 in gp_order:
                                        tensor.wait_ge(pool_s,
                                                       gp_order[s2] + 2)
                                    else:
                                        tensor.wait_ge(ve, vemark[f"oh{s2}"])
                                waited_strip[0] = s
                            j = chunk_of(tile)
                            if j > waited_chunk[0]:
                                tensor.wait_ge(ve, vemark[f"he2{j}"])
                                waited_chunk[0] = j
                            last = tensor.matmul(
                                psums[q][32 * tp:32 * tp + 32,
                                         (bk % 8) * W:(bk % 8) * W + W],
                                sb_he2[:, tile, :],
                                sb_oh[:, tile * W:(tile + 1) * W],
                                start=(i_mm == 0),
                                stop=(i_mm == n_mm - 1),
                                tile_position=(0, 32 * tp),
                            )
                            i_mm += 1
                last.then_inc(pe, 1)

    return nc


def run_gnn_v5(feat, distances, radial_params, features_to_use, src, dst,
               trace=False, tmpdir=None):
    try:
        r = shard_inputs_v5(feat, distances, radial_params, features_to_use,
                            src, dst)
    except Exception:
        r = None
    if r is None:
        return run_gnn_v4(feat, distances, radial_params, features_to_use,
                          src, dst, trace=trace, tmpdir=tmpdir)
    in_maps, plan = r
    nc = build_nc_v5(plan)
    nc.compile()
    res = run_bass_kernel_spmd(nc, in_maps, core_ids=list(range(NCORE)),
                               trace=trace, tmpdir=tmpdir)
    n_nodes = np.asarray(feat).shape[0]
    TK = 2 * plan["K"] * plan["NTP"]
    out = np.zeros((n_nodes, TK), np.float32)
    for c in range(NCORE):
        outT = res.results[c]["outT"]          # [128, NBLK*W] bf16
        out[c * NPC:(c + 1) * NPC, :] = outT.T[:NPC, :].astype(np.float32)
    return out, res


run_gnn_v2 = run_gnn_v5    # back-compat alias for test.py


def kernel(**inputs):
    out, _res = run_gnn_v5(**inputs)
    return out



# revision 9
# speedup vs baseline: 1.0633x; 1.0633x over previous
"""AtomicConv GNN message passing kernel for 8 TRN2 NeuronCores.

out[n, t*K+k] = sum_{e: dst[e]=n} (feat[src[e]] == ftu[t]) * he[e, k]
with the reference's raw (K,E,1)->(E,K) reshape quirk:
  he[e, k] = f_{e//ME}( d[K*(e%ME)+k] ),  ME = E//K
  f_j(x) = exp(-s_j (x-mu_j)^2) * 0.5*(cos(pi*x/c_j)+1) * (x <= c_j)

Sharding strategy: edge-parallel, partitioned by destination-node range
(2500 nodes per core, no collective needed), with edges grouped into
(node-block, type-pair) chunks of 128-edge tiles.  Each core computes the
radial basis (ACT: sin/square/exp; DVE: products & masks), builds
destination one-hots (bulk DVE compares), and performs the segment-sum as
PSUM-accumulated TensorE matmuls over a 10-stage software pipeline,
writing its node range of the output.
"""
import sys
for p in ('/opt/trn_rl_repo', '/root/.axon_site/_ro/trn_rl_repo'):
    if p not in sys.path:
        sys.path.append(p)
from contextlib import ExitStack

import numpy as np
import ml_dtypes

import concourse.bass as bass
import concourse.bacc as bacc
import concourse.mybir as mybir
from concourse.bass_utils import run_bass_kernel_spmd

F32 = mybir.dt.float32
BF16 = mybir.dt.bfloat16
BF16_NP = ml_dtypes.bfloat16

NCORE = 8
NPC = 2500          # real nodes per core
NPAD = 2560         # padded node range: 20 blocks x 128
NBLK = 20
PI = float(np.pi)
PAD_DIST = 50.0
NRANGE = 10


def shard_inputs(feat, distances, radial_params, features_to_use, src, dst):
    feat = np.asarray(feat, np.float32).reshape(-1)
    d = np.asarray(distances, np.float32).reshape(-1)
    rp = np.asarray(radial_params, np.float32)
    ftu = np.asarray(features_to_use, np.float32).reshape(-1)
    src = np.asarray(src).reshape(-1)
    dst = np.asarray(dst).reshape(-1)
    T = ftu.shape[0]
    K = rp.shape[0]
    E = d.shape[0]
    ME = E // K
    assert T % 2 == 0 and E % K == 0
    NTP = T // 2

    fsrc = feat[src]
    eis, ets = [], []
    for t in range(T):
        sel = np.nonzero(fsrc == ftu[t])[0]
        eis.append(sel)
        ets.append(np.full(sel.shape, t, np.int64))
    ei = np.concatenate(eis)            # original edge ids, typed only
    et = np.concatenate(ets)
    edst = dst[ei].astype(np.int64)

    core = edst // NPC
    dstl = edst - core * NPC
    blk = dstl >> 7
    nl = dstl & 127
    tp = et >> 1
    pb = (et & 1).astype(np.float32)
    NCHUNK = NBLK * NTP
    chunk = blk * NTP + tp

    counts = np.zeros((NCORE, NCHUNK), np.int64)
    np.add.at(counts, (core, chunk), 1)
    ntiles = np.maximum(1, -(-counts.max(axis=0) // 128))     # per chunk
    tile_base = np.zeros(NCHUNK, np.int64)
    tile_base[1:] = np.cumsum(ntiles)[:-1]
    S = int(ntiles.sum())

    # rank of each edge within its (core, chunk) group
    key = core * NCHUNK + chunk
    order = np.argsort(key, kind='stable')
    sorted_key = key[order]
    starts = np.searchsorted(sorted_key, np.arange(NCORE * NCHUNK))
    rank_sorted = np.arange(len(order)) - starts[sorted_key]
    rank = np.empty(len(order), np.int64)
    rank[order] = rank_sorted
    slot_tile = tile_base[chunk] + (rank >> 7)
    slot_p = rank & 127

    # per-edge payload under the reshape quirk
    kk = ei // ME                       # radial kernel index per edge
    er = ei % ME
    dvec = d[(K * er)[:, None] + np.arange(K)[None, :]]     # (ne, K)
    mu_e = rp[kk, 1]
    negs_e = -rp[kk, 2]
    c_e = rp[kk, 0]

    d3_a = np.full((NCORE, 128, S, K), PAD_DIST, np.float32)
    mu_a = np.zeros((NCORE, 128, S), np.float32)
    negs_a = np.zeros((NCORE, 128, S), np.float32)
    invc_a = np.full((NCORE, 128, S), 1.0, np.float32)
    nl_a = np.zeros((NCORE, 128, S), np.float32)
    pb0_a = np.zeros((NCORE, 128, S), np.float32)
    pb1_a = np.zeros((NCORE, 128, S), np.float32)
    d3_a[core, slot_p, slot_tile] = dvec
    mu_a[core, slot_p, slot_tile] = mu_e
    negs_a[core, slot_p, slot_tile] = negs_e
    invc_a[core, slot_p, slot_tile] = PI / c_e
    nl_a[core, slot_p, slot_tile] = nl
    pb0_a[core, slot_p, slot_tile] = 0.5 * (1.0 - pb)   # 0.5 factor folded in
    pb1_a[core, slot_p, slot_tile] = 0.5 * pb

    iota_a = np.broadcast_to(np.arange(128, dtype=np.float32), (128, 128)).copy()
    uniform_c = float(rp[0, 0]) if np.all(rp[:, 0] == rp[0, 0]) else None

    in_maps = []
    for c in range(NCORE):
        m = {
            "d3": d3_a[c].reshape(128, S * K),
            "mu": mu_a[c], "negs": negs_a[c],
            "nl": nl_a[c].astype(BF16_NP),
            "pb0": pb0_a[c].astype(BF16_NP),
            "pb1": pb1_a[c].astype(BF16_NP),
            "iota": iota_a.astype(BF16_NP),
        }
        if uniform_c is None:
            m["invc"] = invc_a[c]
        in_maps.append(m)
    sched = [(int(c // NTP), int(c % NTP), int(tile_base[c]), int(ntiles[c]))
             for c in range(NCHUNK)]
    return in_maps, sched, S, K, NTP, uniform_c


def build_nc(S, K, NTP, sched, uniform_c):
    """One program for all 8 cores (SPMD, no collectives)."""
    SK = S * K
    uni = uniform_c is not None
    nc = bacc.Bacc(None, target_bir_lowering=False, debug=False)
    p_d3 = nc.declare_dram_parameter("d3", [128, SK], F32, isOutput=False)
    p_mu = nc.declare_dram_parameter("mu", [128, S], F32, isOutput=False)
    p_negs = nc.declare_dram_parameter("negs", [128, S], F32, isOutput=False)
    p_nl = nc.declare_dram_parameter("nl", [128, S], BF16, isOutput=False)
    p_pb0 = nc.declare_dram_parameter("pb0", [128, S], BF16, isOutput=False)
    p_pb1 = nc.declare_dram_parameter("pb1", [128, S], BF16, isOutput=False)
    p_iota = nc.declare_dram_parameter("iota", [128, 128], BF16, isOutput=False)
    if not uni:
        p_invc = nc.declare_dram_parameter("invc", [128, S], F32, isOutput=False)
    p_out = nc.declare_dram_parameter("outT", [128, NPAD], F32, isOutput=True)

    NBANK = (NBLK + 3) // 4   # 4 blocks of 128 nodes per psum bank

    with ExitStack() as es:
        block = es.enter_context(nc.Block())

        def sem(name):
            return es.enter_context(nc.semaphore(name))

        def sbuf(name, shape, dt):
            return es.enter_context(nc.sbuf_tensor(name, shape, dt))

        dma_in = sem("dma_in"); ve = sem("ve"); ac = sem("ac")
        pe = sem("pe"); outc = sem("outc"); dma_out = sem("dma_out")

        sb_d3 = sbuf("sb_d3", [128, SK], F32)     # also reused for the output
        sb_mu = sbuf("sb_mu", [128, S], F32)
        sb_negs = sbuf("sb_negs", [128, S], F32)
        sb_nl = sbuf("sb_nl", [128, S], BF16)
        sb_pb0 = sbuf("sb_pb0", [128, S], BF16)
        sb_pb1 = sbuf("sb_pb1", [128, S], BF16)
        sb_iota = sbuf("sb_iota", [128, 128], BF16)
        sb_invc = None if uni else sbuf("sb_invc", [128, S], F32)
        sb_uw = sbuf("sb_uw", [128, SK], F32)     # u, then w
        sb_vx = sbuf("sb_vx", [128, SK], F32)     # v, then x
        sb_e1 = sbuf("sb_e1", [128, SK], BF16)
        sb_c1 = sbuf("sb_c1", [128, SK], BF16)
        sb_he = sbuf("sb_he", [128, SK], BF16)
        sb_he2 = sbuf("sb_he2", [128, S, 2 * K], BF16)
        sb_oh = sbuf("sb_oh", [128, S, 128], BF16)
        psums = [es.enter_context(nc.psum_tensor(f"psum{q}", [128, 512], F32))
                 for q in range(NBANK)]

        d3v = sb_d3[:, :].rearrange("p (s k) -> p s k", k=K)
        mu_rep = sb_mu[:, :].unsqueeze(-1).to_broadcast([128, S, K])
        negs_rep = sb_negs[:, :].unsqueeze(-1).to_broadcast([128, S, K])
        pb0_rep = sb_pb0[:, :].unsqueeze(-1).to_broadcast([128, S, K])
        pb1_rep = sb_pb1[:, :].unsqueeze(-1).to_broadcast([128, S, K])
        iota_rep = sb_iota[:, :].unsqueeze(1).to_broadcast([128, S, 128])
        nl_rep = sb_nl[:, :].unsqueeze(-1).to_broadcast([128, S, 128])
        invc_rep = (None if uni else
                    sb_invc[:, :].unsqueeze(-1).to_broadcast([128, S, K]))

        uwv = sb_uw[:, :].rearrange("p (s k) -> p s k", k=K)
        vxv = sb_vx[:, :].rearrange("p (s k) -> p s k", k=K)
        e1v = sb_e1[:, :].rearrange("p (s k) -> p s k", k=K)
        c1v = sb_c1[:, :].rearrange("p (s k) -> p s k", k=K)
        hev = sb_he[:, :].rearrange("p (s k) -> p s k", k=K)
        out_view = sb_d3[:, 0:NPAD]               # reuse d3 buffer for output

        dmas = [(sb_d3, p_d3), (sb_mu, p_mu), (sb_negs, p_negs), (sb_nl, p_nl),
                (sb_pb0, p_pb0), (sb_pb1, p_pb1), (sb_iota, p_iota)]
        if not uni:
            dmas.append((sb_invc, p_invc))
        n_dma_in = len(dmas) * 16

        vemark = {}

        @block.sync
        def _(sync):
            for sb, pr in dmas:
                sync.dma_start(out=sb[:, :], in_=pr[:, :]).then_inc(dma_in, 16)
            sync.wait_ge(outc, NBANK)
            sync.dma_start(out=p_out[:, :], in_=out_view).then_inc(dma_out, 16)
            sync.wait_ge(dma_out, 16)

        @block.vector
        def _(vector):
            cnt = [0]

            def fin(inst, mark=None):
                cnt[0] += 1
                inst.then_inc(ve, 1)
                if mark:
                    vemark[mark] = cnt[0]

            vector.wait_ge(dma_in, n_dma_in)
            # u = d - mu
            fin(vector.tensor_tensor(out=uwv, in0=d3v, in1=mu_rep,
                                     op=mybir.AluOpType.subtract), "u")
            # w = v * (-s)   (v = u^2 from ACT; overwrites u)
            vector.wait_ge(ac, 1)
            fin(vector.tensor_tensor(out=uwv, in0=vxv, in1=negs_rep,
                                     op=mybir.AluOpType.mult), "w")
            # x = clamp(pi/2 - d*pi/c, >= -pi/2)  (overwrites v)
            vector.wait_ge(ve, vemark["w"])
            if uni:
                fin(vector.tensor_scalar(out=sb_vx[:, :], in0=sb_d3[:, :],
                                         scalar1=-PI / float(uniform_c),
                                         scalar2=PI / 2.0,
                                         op0=mybir.AluOpType.mult,
                                         op1=mybir.AluOpType.add), "x1")
            else:
                fin(vector.scalar_tensor_tensor(out=vxv, in0=d3v, scalar=-1.0,
                                                in1=invc_rep,
                                                op0=mybir.AluOpType.mult,
                                                op1=mybir.AluOpType.mult), "x0")
                vector.wait_ge(ve, vemark["x0"])
                fin(vector.tensor_scalar(out=sb_vx[:, :], in0=sb_vx[:, :],
                                         scalar1=PI / 2.0, scalar2=None,
                                         op0=mybir.AluOpType.add), "x1")
            vector.wait_ge(ve, vemark["x1"])
            fin(vector.tensor_scalar(out=sb_vx[:, :], in0=sb_vx[:, :],
                                     scalar1=-PI / 2.0, scalar2=None,
                                     op0=mybir.AluOpType.max), "x")
            # he = (c1 + 1) * e1      (c1: ac>=2, e1: ac>=3)
            vector.wait_ge(ac, 3)
            fin(vector.scalar_tensor_tensor(out=hev, in0=c1v, scalar=1.0,
                                            in1=e1v,
                                            op0=mybir.AluOpType.add,
                                            op1=mybir.AluOpType.mult), "he")
            vector.wait_ge(ve, vemark["he"])
            fin(vector.tensor_tensor(out=sb_he2[:, :, 0:K], in0=hev, in1=pb0_rep,
                                     op=mybir.AluOpType.mult), "he2a")
            fin(vector.tensor_tensor(out=sb_he2[:, :, K:2 * K], in0=hev,
                                     in1=pb1_rep,
                                     op=mybir.AluOpType.mult), "he2b")
            fin(vector.tensor_tensor(out=sb_oh[:, :, :], in0=iota_rep, in1=nl_rep,
                                     op=mybir.AluOpType.is_equal), "oh")

        @block.scalar
        def _(scalar):
            scalar.wait_ge(dma_in, n_dma_in)
            scalar.wait_ge(ve, vemark["u"])
            scalar.activation(out=vxv, in_=uwv,
                              func=mybir.ActivationFunctionType.Square
                              ).then_inc(ac, 1)                       # ac=1 (v)
            scalar.wait_ge(ve, vemark["x"])
            scalar.activation(out=sb_c1[:, :], in_=sb_vx[:, :],
                              func=mybir.ActivationFunctionType.Sin
                              ).then_inc(ac, 1)                       # ac=2 (c1)
            scalar.wait_ge(ve, vemark["w"])
            scalar.activation(out=e1v, in_=uwv,
                              func=mybir.ActivationFunctionType.Exp
                              ).then_inc(ac, 1)                       # ac=3 (e1)
            # after matmuls: copy psum -> sbuf (into the retired d3 buffer)
            scalar.wait_ge(pe, 1)
            for q in range(NBANK):
                scalar.activation(out=out_view[:, q * 512:(q + 1) * 512],
                                  in_=psums[q][:, :],
                                  func=mybir.ActivationFunctionType.Copy
                                  ).then_inc(outc, 1)

        @block.tensor
        def _(tensor):
            tensor.wait_ge(ve, vemark["oh"])
            last = None
            for q in range(NBANK):
                for tp in range(4):
                    grp = [c for c in sched if c[0] // 4 == q and c[1] == tp]
                    n_mm = sum(c[3] for c in grp)
                    i_mm = 0
                    for (b, _tp, tbase, nt) in grp:
                        for i in range(nt):
                            tile = tbase + i
                            last = tensor.matmul(
                                psums[q][32 * tp:32 * tp + 32,
                                         (b % 4) * 128:(b % 4) * 128 + 128],
                                sb_he2[:, tile, :],
                                sb_oh[:, tile, :],
                                start=(i_mm == 0),
                                stop=(i_mm == n_mm - 1),
                                tile_position=(0, 32 * tp),
                            )
                            i_mm += 1
            last.then_inc(pe, 1)

    return nc


def run_gnn(feat, distances, radial_params, features_to_use, src, dst,
            trace=False, tmpdir=None):
    in_maps, sched, S, K, NTP, uniform_c = shard_inputs(
        feat, distances, radial_params, features_to_use, src, dst)
    nc = build_nc(S, K, NTP, sched, uniform_c)
    nc.compile()
    res = run_bass_kernel_spmd(nc, in_maps, core_ids=list(range(NCORE)),
                               trace=trace, tmpdir=tmpdir)
    n_nodes = np.asarray(feat).shape[0]
    TK = 2 * K * NTP
    out = np.zeros((n_nodes, TK), np.float32)
    for c in range(NCORE):
        outT = res.results[c]["outT"]          # [128, NPAD]
        out[c * NPC:(c + 1) * NPC, :] = outT.T[:NPC, :].astype(np.float32)
    return out, res


def shard_inputs_v4(feat, distances, radial_params, features_to_use, src, dst):
    feat = np.asarray(feat, np.float32).reshape(-1)
    d = np.asarray(distances, np.float32).reshape(-1)
    rp = np.asarray(radial_params, np.float32)
    ftu = np.asarray(features_to_use, np.float32).reshape(-1)
    src = np.asarray(src).reshape(-1)
    dst = np.asarray(dst).reshape(-1)
    T = ftu.shape[0]
    K = rp.shape[0]
    E = d.shape[0]
    ME = E // K
    NTP = T // 2

    uniform_c = float(rp[0, 0]) if np.all(rp[:, 0] == rp[0, 0]) else None
    fast = (uniform_c is not None and T % 2 == 0 and E % K == 0
            and float(d.max()) <= uniform_c and float(d.min()) >= 0.0
            and NBLK * NTP % 4 == 0)
    if not fast:
        return None

    fsrc = feat[src]
    eis, ets = [], []
    for t in range(T):
        sel = np.nonzero(fsrc == ftu[t])[0]
        eis.append(sel)
        ets.append(np.full(sel.shape, t, np.int64))
    ei = np.concatenate(eis)
    et = np.concatenate(ets)
    edst = dst[ei].astype(np.int64)

    core = edst // NPC
    dstl = edst - core * NPC
    blk = dstl >> 7
    nl = dstl & 127
    tp = et >> 1
    pb = (et & 1).astype(np.float32)
    NCHUNK = NBLK * NTP
    chunk = blk * NTP + tp

    counts = np.zeros((NCORE, NCHUNK), np.int64)
    np.add.at(counts, (core, chunk), 1)
    ntiles = np.maximum(1, -(-counts.max(axis=0) // 128))
    tile_base = np.zeros(NCHUNK, np.int64)
    tile_base[1:] = np.cumsum(ntiles)[:-1]
    S = int(ntiles.sum())

    key = core * NCHUNK + chunk
    order = np.argsort(key, kind='stable')
    sorted_key = key[order]
    starts = np.searchsorted(sorted_key, np.arange(NCORE * NCHUNK))
    rank_sorted = np.arange(len(order)) - starts[sorted_key]
    rank = np.empty(len(order), np.int64)
    rank[order] = rank_sorted
    slot_tile = tile_base[chunk] + (rank >> 7)
    slot_p = rank & 127

    kk = ei // ME
    er = ei % ME
    dvec = d[(K * er)[:, None] + np.arange(K)[None, :]]
    sq_e = np.sqrt(rp[kk, 2])
    dvs = (dvec - rp[kk, 1][:, None]) * sq_e[:, None]   # sqrt(s)*(d-mu)

    pad_d = 1.4925 * uniform_c
    d3_a = np.full((NCORE, 128, S, K), pad_d, np.float32)
    d3s_a = np.full((NCORE, 128, S, K), 50.0, np.float32)
    nl_a = np.zeros((NCORE, 128, S), np.float32)
    pb0_a = np.zeros((NCORE, 128, S), np.float32)
    pb1_a = np.zeros((NCORE, 128, S), np.float32)
    d3_a[core, slot_p, slot_tile] = dvec
    d3s_a[core, slot_p, slot_tile] = dvs
    nl_a[core, slot_p, slot_tile] = nl
    pb0_a[core, slot_p, slot_tile] = 0.5 * (1.0 - pb)
    pb1_a[core, slot_p, slot_tile] = 0.5 * pb

    iota_a = np.broadcast_to(np.arange(128, dtype=np.float32), (128, 128)).copy()
    pb0e = np.repeat(pb0_a, K, axis=2).astype(BF16_NP)   # [NCORE,128,S*K]
    pb1e = np.repeat(pb1_a, K, axis=2).astype(BF16_NP)

    in_maps = []
    for c in range(NCORE):
        in_maps.append({
            "d3": d3_a[c].reshape(128, S * K),
            "d3s": d3s_a[c].reshape(128, S * K),
            "nl": nl_a[c],
            "pb0e": pb0e[c], "pb1e": pb1e[c],
            "iota": iota_a.astype(BF16_NP),
            "hpi": np.full((128, 1), PI / 2.0, np.float32),
        })
    sched = [(int(c // NTP), int(c % NTP), int(tile_base[c]), int(ntiles[c]))
             for c in range(NCHUNK)]
    return in_maps, sched, S, K, NTP, uniform_c


def build_nc_v4(S, K, NTP, sched, uniform_c):
    SK = S * K
    nc = bacc.Bacc(None, target_bir_lowering=False, debug=False)
    p_d3 = nc.declare_dram_parameter("d3", [128, SK], F32, isOutput=False)
    p_d3s = nc.declare_dram_parameter("d3s", [128, SK], F32, isOutput=False)
    p_nl = nc.declare_dram_parameter("nl", [128, S], F32, isOutput=False)
    p_pb0e = nc.declare_dram_parameter("pb0e", [128, SK], BF16, isOutput=False)
    p_pb1e = nc.declare_dram_parameter("pb1e", [128, SK], BF16, isOutput=False)
    p_iota = nc.declare_dram_parameter("iota", [128, 128], BF16, isOutput=False)
    p_hpi = nc.declare_dram_parameter("hpi", [128, 1], F32, isOutput=False)
    p_out = nc.declare_dram_parameter("outT", [128, NPAD], F32, isOutput=True)

    NBANK = NBLK // 4
    # tile ranges for the DVE/ACT pipeline
    rb = [round(S * r / NRANGE) for r in range(NRANGE + 1)]
    ranges = [(rb[r], rb[r + 1]) for r in range(NRANGE)]
    # bank tile spans (chunk ids are contiguous per bank)
    bank_span = []
    for q in range(NBANK):
        lo = sched[16 * q][2]
        hi_c = sched[16 * q + 15]
        bank_span.append((lo, hi_c[2] + hi_c[3]))
    # last pipeline range needed per bank
    bank_need_range = [max(r for r in range(NRANGE) if ranges[r][0] < hi)
                       for (lo, hi) in bank_span]

    OHB = 6                     # one-hot rotating buffer (in units of ranges)
    # last bank whose tile span covers range j (for oh slot-reuse gating)
    range_last_bank = [max(q for q in range(NBANK)
                           if bank_span[q][0] < ranges[j][1])
                       for j in range(NRANGE)]
    max_rt = max(b - a for a, b in ranges)

    with ExitStack() as es:
        block = es.enter_context(nc.Block())

        def sem(name):
            return es.enter_context(nc.semaphore(name))

        def sbuf(name, shape, dt):
            return es.enter_context(nc.sbuf_tensor(name, shape, dt))

        dma_in = sem("dma_in"); ve = sem("ve"); ac = sem("ac")
        pe = sem("pe"); outc = sem("outc"); dma_out = sem("dma_out")
        dma_d3 = [sem(f"dma_d3_{r}") for r in range(NRANGE)]
        dma_ds = [sem(f"dma_ds_{r}") for r in range(NRANGE)]
        dma_pb = [sem(f"dma_pb_{r}") for r in range(NRANGE)]

        sb_d3 = sbuf("sb_d3", [128, SK], F32)
        sb_d3s = sbuf("sb_d3s", [128, SK], F32)
        sb_nl = sbuf("sb_nl", [128, S], F32)
        sb_pb0e = sbuf("sb_pb0e", [128, SK], BF16)
        sb_pb1e = sbuf("sb_pb1e", [128, SK], BF16)
        sb_iota = sbuf("sb_iota", [128, 128], BF16)
        sb_hpi = sbuf("sb_hpi", [128, 1], F32)
        sb_v = sbuf("sb_v", [128, SK], F32)
        sb_e1 = sbuf("sb_e1", [128, SK], BF16)
        sb_c1 = sbuf("sb_c1", [128, SK], BF16)
        sb_ht = sbuf("sb_ht", [128, SK], BF16)
        sb_he2 = sbuf("sb_he2", [128, S, 2 * K], BF16)
        sb_oh = sbuf("sb_oh", [128, OHB, max_rt * 128], BF16)
        sb_out = sbuf("sb_out", [128, NPAD], F32)
        psums = [es.enter_context(nc.psum_tensor(f"psum{q}", [128, 512], F32))
                 for q in range(NBANK)]

        # DMA plan: small tensors first, then per-range d3/pb0e/pb1e
        small_dmas = [(sb_nl, p_nl), (sb_iota, p_iota), (sb_hpi, p_hpi)]
        n_small = len(small_dmas)
        # dma_in counts: small: 16 each; then per range r: 3 DMAs
        def dma_mark_small():
            return 16 * n_small

        def dma_mark_range(r):
            return 16 * n_small + 48 * (r + 1)

        vemark = {}
        acmark = {}

        @block.sync
        def _(sync):
            for sb, pr in small_dmas:
                sync.dma_start(out=sb[:, :], in_=pr[:, :]).then_inc(dma_in, 16)
            for r, (a, b) in enumerate(ranges):
                ka, kb = a * K, b * K
                sync.dma_start(out=sb_d3[:, ka:kb],
                               in_=p_d3[:, ka:kb]).then_inc(dma_d3[r], 16)
            for r, (a, b) in enumerate(ranges):
                ka, kb = a * K, b * K
                sync.dma_start(out=sb_d3s[:, ka:kb],
                               in_=p_d3s[:, ka:kb]).then_inc(dma_ds[r], 16)
            for r, (a, b) in enumerate(ranges):
                ka, kb = a * K, b * K
                sync.dma_start(out=sb_pb0e[:, ka:kb],
                               in_=p_pb0e[:, ka:kb]).then_inc(dma_pb[r], 16)
                sync.dma_start(out=sb_pb1e[:, ka:kb],
                               in_=p_pb1e[:, ka:kb]).then_inc(dma_pb[r], 16)
            # output: per bank as soon as copied
            for q in range(NBANK):
                sync.wait_ge(outc, q + 1)
                sync.dma_start(out=p_out[:, q * 512:(q + 1) * 512],
                               in_=sb_out[:, q * 512:(q + 1) * 512]
                               ).then_inc(dma_out, 16)
            sync.wait_ge(dma_out, 16 * NBANK)

        @block.vector
        def _(vector):
            cnt = [0]

            def fin(inst, mark=None):
                cnt[0] += 1
                inst.then_inc(ve, 1)
                if mark:
                    vemark[mark] = cnt[0]

            def emit_oh(r):
                a, b = ranges[r]
                nt = b - a
                par = r % OHB
                if r >= OHB:
                    vector.wait_ge(pe, range_last_bank[r - OHB] + 1)
                fin(vector.tensor_tensor(
                    out=sb_oh[:, par, 0:nt * 128].rearrange(
                        "p (t n) -> p t n", n=128),
                    in0=sb_iota[:, :].unsqueeze(1).to_broadcast([128, nt, 128]),
                    in1=sb_nl[:, a:b].unsqueeze(-1).to_broadcast([128, nt, 128]),
                    op=mybir.AluOpType.is_equal))
                vemark[f"oh{r}"] = cnt[0]

            def emit_chain(r):
                a, b = ranges[r]
                ka, kb = a * K, b * K
                # c1p1 = c1 + 1 (in place), needs Sin_r  (ac >= r+1)
                vector.wait_ge(ac, r + 1)
                fin(vector.tensor_scalar(out=sb_c1[:, ka:kb],
                                         in0=sb_c1[:, ka:kb],
                                         scalar1=1.0, scalar2=None,
                                         op0=mybir.AluOpType.add), f"c1p1{r}")
                # ht = c1p1 * e1, needs Exp_r  (ac >= NRANGE + 2r + 2)
                vector.wait_ge(ve, vemark[f"c1p1{r}"])
                vector.wait_ge(ac, NRANGE + 2 * r + 2)
                fin(vector.tensor_tensor(out=sb_ht[:, ka:kb],
                                         in0=sb_c1[:, ka:kb],
                                         in1=sb_e1[:, ka:kb],
                                         op=mybir.AluOpType.mult), f"ht{r}")
                vector.wait_ge(ve, vemark[f"ht{r}"])
                vector.wait_ge(dma_pb[r], 32)
                fin(vector.tensor_tensor(out=sb_he2[:, a:b, 0:K],
                                         in0=sb_ht[:, ka:kb].rearrange(
                                             "p (s k) -> p s k", k=K),
                                         in1=sb_pb0e[:, ka:kb].rearrange(
                                             "p (s k) -> p s k", k=K),
                                         op=mybir.AluOpType.mult), f"he2a{r}")
                fin(vector.tensor_tensor(out=sb_he2[:, a:b, K:2 * K],
                                         in0=sb_ht[:, ka:kb].rearrange(
                                             "p (s k) -> p s k", k=K),
                                         in1=sb_pb1e[:, ka:kb].rearrange(
                                             "p (s k) -> p s k", k=K),
                                         op=mybir.AluOpType.mult), f"he2b{r}")

            vector.wait_ge(dma_in, dma_mark_small())
            for r in range(NRANGE):
                emit_oh(r)
                if r >= 1:
                    emit_chain(r - 1)
            emit_chain(NRANGE - 1)

        @block.scalar
        def _(scalar):
            scalar.wait_ge(dma_in, dma_mark_small())
            # pass 1: all Sins (trig table loaded once)    ac = r+1
            for r, (a, b) in enumerate(ranges):
                ka, kb = a * K, b * K
                scalar.wait_ge(dma_d3[r], 16)
                scalar.activation(out=sb_c1[:, ka:kb], in_=sb_d3[:, ka:kb],
                                  func=mybir.ActivationFunctionType.Sin,
                                  bias=sb_hpi[:, 0:1],
                                  scale=-PI / float(uniform_c)).then_inc(ac, 1)
            # pass 2: Square + Exp per range (exp table)   ac = NRANGE+2r+1, +2
            for r, (a, b) in enumerate(ranges):
                ka, kb = a * K, b * K
                scalar.wait_ge(dma_ds[r], 16)
                scalar.activation(out=sb_v[:, ka:kb], in_=sb_d3s[:, ka:kb],
                                  func=mybir.ActivationFunctionType.Square
                                  ).then_inc(ac, 1)
                scalar.wait_ge(ac, NRANGE + 2 * r + 1)
                scalar.activation(out=sb_e1[:, ka:kb], in_=sb_v[:, ka:kb],
                                  func=mybir.ActivationFunctionType.Exp,
                                  scale=-1.0).then_inc(ac, 1)
            for q in range(NBANK):
                scalar.wait_ge(pe, q + 1)
                scalar.activation(out=sb_out[:, q * 512:(q + 1) * 512],
                                  in_=psums[q][:, :],
                                  func=mybir.ActivationFunctionType.Copy
                                  ).then_inc(outc, 1)

        @block.tensor
        def _(tensor):
            # range r tiles live in parity slot r%OHB of sb_oh
            def oh_ap(tile):
                r = next(i for i, (a, b) in enumerate(ranges)
                         if a <= tile < b)
                a, _ = ranges[r]
                i = tile - a
                return sb_oh[:, r % OHB, i * 128:(i + 1) * 128], r

            for q in range(NBANK):
                # wait for all DVE work covering this bank's tiles
                need_r = bank_need_range[q]
                tensor.wait_ge(ve, vemark[f"he2b{need_r}"])
                for tp in range(4):
                    grp = [c for c in sched if c[0] // 4 == q and c[1] == tp]
                    n_mm = sum(c[3] for c in grp)
                    i_mm = 0
                    for (bk, _tp, tbase, nt) in grp:
                        for i in range(nt):
                            tile = tbase + i
                            ohap, _r = oh_ap(tile)
                            last = tensor.matmul(
                                psums[q][32 * tp:32 * tp + 32,
                                         (bk % 4) * 128:(bk % 4) * 128 + 128],
                                sb_he2[:, tile, :],
                                ohap,
                                start=(i_mm == 0),
                                stop=(i_mm == n_mm - 1),
                                tile_position=(0, 32 * tp),
                            )
                            i_mm += 1
                last.then_inc(pe, 1)

    return nc


def run_gnn_v4(feat, distances, radial_params, features_to_use, src, dst,
               trace=False, tmpdir=None):
    r = shard_inputs_v4(feat, distances, radial_params, features_to_use,
                        src, dst)
    if r is None:
        return run_gnn(feat, distances, radial_params, features_to_use,
                       src, dst, trace=trace, tmpdir=tmpdir)
    in_maps, sched, S, K, NTP, uniform_c = r
    nc = build_nc_v4(S, K, NTP, sched, uniform_c)
    nc.compile()
    res = run_bass_kernel_spmd(nc, in_maps, core_ids=list(range(NCORE)),
                               trace=trace, tmpdir=tmpdir)
    n_nodes = np.asarray(feat).shape[0]
    TK = 2 * K * NTP
    out = np.zeros((n_nodes, TK), np.float32)
    for c in range(NCORE):
        outT = res.results[c]["outT"]
        out[c * NPC:(c + 1) * NPC, :] = outT.T[:NPC, :].astype(np.float32)
    return out, res


def shard_inputs_v5(feat, distances, radial_params, features_to_use, src, dst,
                    W=64, NG=10):
    """v5: W-wide node blocks, fp16 d3 + host-squared gaussian arg,
    one-hot strips split between GPSIMD local_scatter and DVE is_equal."""
    feat = np.asarray(feat, np.float32).reshape(-1)
    d = np.asarray(distances, np.float32).reshape(-1)
    rp = np.asarray(radial_params, np.float32)
    ftu = np.asarray(features_to_use, np.float32).reshape(-1)
    src = np.asarray(src).reshape(-1)
    dst = np.asarray(dst).reshape(-1)
    T = ftu.shape[0]
    K = rp.shape[0]
    E = d.shape[0]
    if K == 0 or E % K != 0 or T % 2 != 0:
        return None
    ME = E // K
    NTP = T // 2

    uniform_c = float(rp[0, 0]) if np.all(rp[:, 0] == rp[0, 0]) else None
    if (uniform_c is None or float(d.max()) > uniform_c
            or float(d.min()) < 0.0):
        return None
    NBLK = -(-NPC // W)
    if NBLK % 8 != 0:
        return None
    NBANK = NBLK // 8

    fsrc = feat[src]
    eis, ets = [], []
    for t in range(T):
        sel = np.nonzero(fsrc == ftu[t])[0]
        eis.append(sel)
        ets.append(np.full(sel.shape, t, np.int64))
    ei = np.concatenate(eis)
    et = np.concatenate(ets)
    edst = dst[ei].astype(np.int64)

    core = edst // NPC
    dstl = edst - core * NPC
    tp = et >> 1
    pb = (et & 1).astype(np.float32)

    # Bin-pack nodes into W-wide blocks, balancing per-(block, tp) edge
    # counts so chunk tile counts stay at ceil(mean/128).  The one-hot
    # column of a node is a free choice; the host unpermutes the output.
    cnts = np.zeros((NCORE, NPC, NTP), np.int64)
    np.add.at(cnts, (core, dstl, tp), 1)
    blk_of = np.zeros((NCORE, NPC), np.int64)
    pos_of = np.zeros((NCORE, NPC), np.int64)
    for c in range(NCORE):
        v = cnts[c]
        order_n = np.argsort(-v.sum(1), kind='stable')
        loads = np.zeros((NBLK, NTP), np.int64)
        fill = np.zeros(NBLK, np.int64)
        for n in order_n:
            cand = np.nonzero(fill < W)[0]
            b = cand[np.argmin((loads[cand] + v[n]).max(1))]
            blk_of[c, n] = b
            pos_of[c, n] = fill[b]
            loads[b] += v[n]
            fill[b] += 1
    blk = blk_of[core, dstl]
    nl = pos_of[core, dstl]
    NCHUNK = NBLK * NTP
    chunk = blk * NTP + tp

    counts = np.zeros((NCORE, NCHUNK), np.int64)
    np.add.at(counts, (core, chunk), 1)
    ntiles = np.maximum(1, -(-counts.max(axis=0) // 128))
    S = int(ntiles.sum())
    if S % 2 == 1:              # even S so strip sizes stay even
        ntiles[-1] += 1
        S += 1

    tile_base = np.zeros(NCHUNK, np.int64)
    tile_base[1:] = np.cumsum(ntiles)[:-1]

    key = core * NCHUNK + chunk
    order = np.argsort(key, kind='stable')
    sorted_key = key[order]
    starts = np.searchsorted(sorted_key, np.arange(NCORE * NCHUNK))
    rank_sorted = np.arange(len(order)) - starts[sorted_key]
    rank = np.empty(len(order), np.int64)
    rank[order] = rank_sorted
    slot_tile = tile_base[chunk] + (rank >> 7)
    slot_p = rank & 127

    # one-hot strip grid (gpsimd local_scatter limit: nt*W*32 < 2^16)
    nt_cap = (2 ** 16 // 32 // W) & ~1
    NSTRIP = max(16, -(-S // nt_cap))
    nt_strip = -(-S // NSTRIP)
    nt_strip += nt_strip % 2
    NSTRIP = -(-S // nt_strip)
    rb = [min(S, i * nt_strip) for i in range(NSTRIP + 1)]
    dve_strips = [s_ for s_ in range(NSTRIP) if s_ % 5 == 4]
    gp_strips = [s_ for s_ in range(NSTRIP) if s_ % 5 != 4]
    NACT = 6
    fr = [0.0, 0.20, 0.40, 0.60, 0.78, 0.91, 1.0]
    cb = [int(round(S * f)) for f in fr]
    sin_groups = [[0], [1], [2, 3, 4, 5]]
    sin_of_chunk = [1, 2, 3, 3, 3, 3]

    # per-edge payload under the reshape quirk
    kk = ei // ME
    er = ei % ME
    dvec = d[(K * er)[:, None] + np.arange(K)[None, :]]     # (ne, K)
    dsq = (rp[kk, 2][:, None] * (dvec - rp[kk, 1][:, None]) ** 2)

    d3_a = np.full((NCORE, 128, S, K), 1.4925 * uniform_c, np.float32)
    dsq_a = np.full((NCORE, 128, S, K), 30000.0, np.float32)
    nl_a = np.zeros((NCORE, 128, S), np.float32)
    pb0_a = np.zeros((NCORE, 128, S), np.float32)
    pb1_a = np.zeros((NCORE, 128, S), np.float32)
    ohidx_a = np.full((NCORE, 128, S), -1, np.int16)
    d3_a[core, slot_p, slot_tile] = dvec
    dsq_a[core, slot_p, slot_tile] = dsq
    nl_a[core, slot_p, slot_tile] = nl
    pb0_a[core, slot_p, slot_tile] = 1.0 - pb
    pb1_a[core, slot_p, slot_tile] = pb
    strip_of_slot = np.minimum(slot_tile // nt_strip, NSTRIP - 1)
    ohidx_a[core, slot_p, slot_tile] = (
        (slot_tile - strip_of_slot * nt_strip) * W + nl).astype(np.int16)

    iota_a = np.broadcast_to(np.arange(W, dtype=np.float32), (128, W)).copy()
    pb0e = np.repeat(pb0_a, K, axis=2).astype(BF16_NP)

    in_maps = []
    for c in range(NCORE):
        in_maps.append({
            "d3": d3_a[c].reshape(128, S * K).astype(np.float16),
            "dsq": dsq_a[c].reshape(128, S * K).astype(np.float16),
            "nl": nl_a[c].astype(BF16_NP),
            "ohidx": ohidx_a[c],
            "pb0e": pb0e[c],
            "iota": iota_a.astype(BF16_NP),
            "hpi": np.full((128, 1), PI / 2.0, np.float32),
        })
    sched = [(int(c // NTP), int(c % NTP), int(tile_base[c]), int(ntiles[c]))
             for c in range(NCHUNK)]
    node_col = blk_of * W + pos_of          # [NCORE, NPC]
    plan = dict(S=S, K=K, NTP=NTP, W=W, NBLK=NBLK, NBANK=NBANK,
                sched=sched, rb=rb, cb=cb, NSTRIP=NSTRIP, NACT=NACT,
                dve_strips=dve_strips, gp_strips=gp_strips,
                sin_groups=sin_groups, sin_of_chunk=sin_of_chunk,
                nt_strip=nt_strip, uniform_c=uniform_c,
                node_col=node_col)
    return in_maps, plan


def build_nc_v5(plan):
    S = plan["S"]; K = plan["K"]; NTP = plan["NTP"]; W = plan["W"]
    NBLK = plan["NBLK"]; NBANK = plan["NBANK"]; sched = plan["sched"]
    rb = plan["rb"]; cb = plan["cb"]; NSTRIP = plan["NSTRIP"]
    NACT = plan["NACT"]; nt_strip = plan["nt_strip"]
    dve_strips = plan["dve_strips"]; gp_strips = plan["gp_strips"]
    sin_groups = plan["sin_groups"]; sin_of_chunk = plan["sin_of_chunk"]
    uniform_c = plan["uniform_c"]
    SK = S * K
    NPADW = NBLK * W
    FP16 = mybir.dt.float16
    I16 = mybir.dt.int16

    nc = bacc.Bacc(None, target_bir_lowering=False, debug=False)
    p_d3 = nc.declare_dram_parameter("d3", [128, SK], FP16, isOutput=False)
    p_dsq = nc.declare_dram_parameter("dsq", [128, SK], FP16, isOutput=False)
    p_nl = nc.declare_dram_parameter("nl", [128, S], BF16, isOutput=False)
    p_ohidx = nc.declare_dram_parameter("ohidx", [128, S], I16, isOutput=False)
    p_pb0e = nc.declare_dram_parameter("pb0e", [128, SK], BF16, isOutput=False)
    p_iota = nc.declare_dram_parameter("iota", [128, W], BF16, isOutput=False)
    p_hpi = nc.declare_dram_parameter("hpi", [128, 1], F32, isOutput=False)
    p_out = nc.declare_dram_parameter("outT", [128, NPADW], BF16, isOutput=True)


    with ExitStack() as es:
        block = es.enter_context(nc.Block())

        def sem(name):
            return es.enter_context(nc.semaphore(name))

        def sbuf(name, shape, dt):
            return es.enter_context(nc.sbuf_tensor(name, shape, dt))

        dma_hd = sem("dma_hd")
        dma_d3 = [sem(f"dma_d3_{j}") for j in range(NACT)]
        dma_dq = [sem(f"dma_dq_{j}") for j in range(NACT)]
        dma_pb = [sem(f"dma_pb_{j}") for j in range(NACT)]
        sin_s = sem("sin_s"); exp_s = sem("exp_s")
        ve = sem("ve"); pool_s = sem("pool_s")
        pe = sem("pe"); outc = sem("outc"); dma_out = sem("dma_out")

        sb_d3 = sbuf("sb_d3", [128, SK], FP16)
        sb_dsq = sbuf("sb_dsq", [128, SK], FP16)
        sb_c1 = sbuf("sb_c1", [128, SK], BF16)      # sin result, then ht
        sb_e1 = sbuf("sb_e1", [128, SK], BF16)
        sb_pb0e = sbuf("sb_pb0e", [128, SK], BF16)
        sb_he2 = sbuf("sb_he2", [128, S, 2 * K], BF16)
        sb_oh = sbuf("sb_oh", [128, S * W], BF16)
        sb_nl = sbuf("sb_nl", [128, S], BF16)
        sb_ohidx = sbuf("sb_ohidx", [128, S], I16)
        sb_iota = sbuf("sb_iota", [128, W], BF16)
        sb_ones = sbuf("sb_ones", [128, nt_strip], BF16)
        sb_warm = sbuf("sb_warm", [128, 2], BF16)
        sb_warmi = sbuf("sb_warmi", [128, 2], I16)
        sb_hpi = sbuf("sb_hpi", [128, 1], F32)
        sb_out = sbuf("sb_out", [128, NPADW], BF16)
        psums = [es.enter_context(nc.psum_tensor(f"psum{q}", [128, 512], F32))
                 for q in range(NBANK)]

        vemark = {}

        # ---- DMA in: head, then per-act-chunk interleaved ----
        @block.sync
        def _(sync):
            sync.dma_start(out=sb_ohidx[:, :], in_=p_ohidx[:, :]).then_inc(dma_hd, 16)
            sync.dma_start(out=sb_hpi[:, :], in_=p_hpi[:, :]).then_inc(dma_hd, 16)

            def dma_d3j(j):
                ka, kb = cb[j] * K, cb[j + 1] * K
                sync.dma_start(out=sb_d3[:, ka:kb],
                               in_=p_d3[:, ka:kb]).then_inc(dma_d3[j], 16)

            def dma_dqj(j):
                ka, kb = cb[j] * K, cb[j + 1] * K
                sync.dma_start(out=sb_dsq[:, ka:kb],
                               in_=p_dsq[:, ka:kb]).then_inc(dma_dq[j], 16)

            def dma_pbj(j):
                ka, kb = cb[j] * K, cb[j + 1] * K
                sync.dma_start(out=sb_pb0e[:, ka:kb],
                               in_=p_pb0e[:, ka:kb]).then_inc(dma_pb[j], 16)

            dma_d3j(0)
            sync.dma_start(out=sb_iota[:, :], in_=p_iota[:, :]).then_inc(dma_hd, 16)
            sync.dma_start(out=sb_nl[:, :], in_=p_nl[:, :]).then_inc(dma_hd, 16)
            dma_d3j(1)
            dma_dqj(0)
            dma_dqj(1)
            dma_pbj(0)
            dma_pbj(1)
            dma_d3j(2)
            dma_d3j(3)
            dma_d3j(4)
            dma_d3j(5)
            for j in range(2, NACT):
                dma_dqj(j)
                dma_pbj(j)
            for q in range(NBANK):
                sync.wait_ge(outc, q + 1)
                sync.dma_start(out=p_out[:, q * 512:(q + 1) * 512],
                               in_=sb_out[:, q * 512:(q + 1) * 512]
                               ).then_inc(dma_out, 16)
            sync.wait_ge(dma_out, 16 * NBANK)

        # ---- DVE: memsets, late one-hot strips, he2 chains ----
        @block.vector
        def _(vector):
            cnt = [0]

            def fin(inst, mark=None):
                cnt[0] += 1
                inst.then_inc(ve, 1)
                if mark:
                    vemark[mark] = cnt[0]

            fin(vector.memset(sb_warmi[:, :], -1))
            fin(vector.memset(sb_ones[:, :], 1.0))          # ve=2: pool warmup
            waited_hd = [False]

            def emit_strip(s):
                if not waited_hd[0]:
                    vector.wait_ge(dma_hd, 64)
                    waited_hd[0] = True
                a, b = rb[s], rb[s + 1]
                nt = b - a
                fin(vector.tensor_tensor(
                    out=sb_oh[:, a * W:b * W].rearrange(
                        "p (t n) -> p t n", n=W),
                    in0=sb_iota[:, :].unsqueeze(1).to_broadcast([128, nt, W]),
                    in1=sb_nl[:, a:b].unsqueeze(-1).to_broadcast([128, nt, W]),
                    op=mybir.AluOpType.is_equal), f"oh{s}")

            def emit_chain(j):
                ka, kb = cb[j] * K, cb[j + 1] * K
                a, b = cb[j], cb[j + 1]
                vector.wait_ge(sin_s, sin_of_chunk[j])
                # c1p1h = 0.5*c1 + 0.5   (tensor_scalar, 4x mode)
                fin(vector.tensor_scalar(
                    out=sb_c1[:, ka:kb], in0=sb_c1[:, ka:kb],
                    scalar1=0.5, scalar2=0.5,
                    op0=mybir.AluOpType.mult, op1=mybir.AluOpType.add))
                vector.wait_ge(exp_s, j + 1)
                # ht = c1p1h * e1, in place over c1
                fin(vector.tensor_tensor(
                    out=sb_c1[:, ka:kb], in0=sb_c1[:, ka:kb],
                    in1=sb_e1[:, ka:kb], op=mybir.AluOpType.mult))
                vector.wait_ge(dma_pb[j], 16)
                fin(vector.tensor_tensor(
                    out=sb_he2[:, a:b, 0:K],
                    in0=sb_c1[:, ka:kb].rearrange("p (s k) -> p s k", k=K),
                    in1=sb_pb0e[:, ka:kb].rearrange("p (s k) -> p s k", k=K),
                    op=mybir.AluOpType.mult))
                fin(vector.tensor_tensor(
                    out=sb_he2[:, a:b, K:2 * K],
                    in0=sb_c1[:, ka:kb].rearrange("p (s k) -> p s k", k=K),
                    in1=sb_he2[:, a:b, 0:K],
                    op=mybir.AluOpType.subtract), f"he2{j}")

            for s in dve_strips:
                emit_strip(s)
            for j in range(NACT):
                emit_chain(j)

        # ---- GPSIMD: warmup (library load), early one-hot strips ----
        @block.gpsimd
        def _(gp):
            gp.wait_ge(ve, 2)
            gp.local_scatter(sb_warm[:, :], sb_ones[:, 0:2], sb_warmi[:, :],
                             channels=128, num_elems=2,
                             num_idxs=2).then_inc(pool_s, 1)
            gp.wait_ge(dma_hd, 16)
            for g in gp_strips:
                a, b = rb[g], rb[g + 1]
                nt = b - a
                gp.local_scatter(sb_oh[:, a * W:b * W], sb_ones[:, 0:nt],
                                 sb_ohidx[:, a:b], channels=128,
                                 num_elems=nt * W,
                                 num_idxs=nt).then_inc(pool_s, 1)

        # ---- ACT: split-phase sin/exp, then psum drains ----
        @block.scalar
        def _(scalar):
            scalar.wait_ge(ve, 2)
            scalar.activation(out=sb_warm[:, 0:1], in_=sb_ones[:, 0:1],
                              func=mybir.ActivationFunctionType.Sin)

            def emit_sin(grp):
                ka, kb = cb[grp[0]] * K, cb[grp[-1] + 1] * K
                scalar.wait_ge(dma_hd, 32)
                for j in grp:
                    scalar.wait_ge(dma_d3[j], 16)
                scalar.activation(out=sb_c1[:, ka:kb], in_=sb_d3[:, ka:kb],
                                  func=mybir.ActivationFunctionType.Sin,
                                  bias=sb_hpi[:, 0:1],
                                  scale=-PI / float(uniform_c)
                                  ).then_inc(sin_s, 1)

            def emit_exp(j):
                ka, kb = cb[j] * K, cb[j + 1] * K
                scalar.wait_ge(dma_dq[j], 16)
                scalar.activation(out=sb_e1[:, ka:kb], in_=sb_dsq[:, ka:kb],
                                  func=mybir.ActivationFunctionType.Exp,
                                  scale=-1.0).then_inc(exp_s, 1)

            emit_sin(sin_groups[0])
            emit_sin(sin_groups[1])
            emit_exp(0)
            emit_exp(1)
            emit_sin(sin_groups[2])
            for j in range(2, NACT):
                emit_exp(j)
            for q in range(NBANK):
                scalar.wait_ge(pe, q + 1)
                scalar.activation(out=sb_out[:, q * 512:(q + 1) * 512],
                                  in_=psums[q][:, :],
                                  func=mybir.ActivationFunctionType.Copy
                                  ).then_inc(outc, 1)

        # ---- TensorE: psum-accumulated scatter matmuls ----
        gp_order = {g: i for i, g in enumerate(gp_strips)}

        @block.tensor
        def _(tensor):
            waited_strip = [-1]
            waited_chunk = [-1]

            def chunk_of(t):
                for j in range(NACT):
                    if t < cb[j + 1]:
                        return j
                return NACT - 1

            for q in range(NBANK):
                last = None
                for tp in range(NTP):
                    grp = [c for c in sched if c[0] // 8 == q and c[1] == tp]
                    n_mm = sum(c[3] for c in grp)
                    i_mm = 0
                    for (bk, _tp, tbase, nt) in grp:
                        for i in range(nt):
                            tile = tbase + i
                            s = min(tile // nt_strip, NSTRIP - 1)
                            if s > waited_strip[0]:
                                for s2 in range(waited_strip[0] + 1, s + 1):
                                    if s2 in gp_order:
                                        tensor.wait_ge(pool_s,
                                                       gp_order[s2] + 2)
                                    else:
                                        tensor.wait_ge(ve, vemark[f"oh{s2}"])
                                waited_strip[0] = s
                            j = chunk_of(tile)
                            if j > waited_chunk[0]:
                                tensor.wait_ge(ve, vemark[f"he2{j}"])
                                waited_chunk[0] = j
                            last = tensor.matmul(
                                psums[q][32 * tp:32 * tp + 32,
                                         (bk % 8) * W:(bk % 8) * W + W],
                                sb_he2[:, tile, :],
                                sb_oh[:, tile * W:(tile + 1) * W],
                                start=(i_mm == 0),
                                stop=(i_mm == n_mm - 1),
                                tile_position=(0, 32 * tp),
                            )
                            i_mm += 1
                last.then_inc(pe, 1)

    return nc


def run_gnn_v5(feat, distances, radial_params, features_to_use, src, dst,
               trace=False, tmpdir=None):
    try:
        r = shard_inputs_v5(feat, distances, radial_params, features_to_use,
                            src, dst)
    except Exception:
        r = None
    if r is None:
        return run_gnn_v4(feat, distances, radial_params, features_to_use,
                          src, dst, trace=trace, tmpdir=tmpdir)
    in_maps, plan = r
    nc = build_nc_v5(plan)
    nc.compile()
    res = run_bass_kernel_spmd(nc, in_maps, core_ids=list(range(NCORE)),
                               trace=trace, tmpdir=tmpdir)
    n_nodes = np.asarray(feat).shape[0]
    TK = 2 * plan["K"] * plan["NTP"]
    node_col = plan["node_col"]
    out = np.zeros((n_nodes, TK), np.float32)
    for c in range(NCORE):
        outT = res.results[c]["outT"]          # [128, NBLK*W] bf16
        out[c * NPC:(c + 1) * NPC, :] = \
            outT.T[node_col[c], :].astype(np.float32)
    return out, res


run_gnn_v2 = run_gnn_v5    # back-compat alias for test.py


def kernel(**inputs):
    out, _res = run_gnn_v5(**inputs)
    return out

